# revision 8
# baseline (speedup 1.0000x reference)
"""Chamfer p=5 loss (nn_ChamferLossP) — Bass kernel for 8x TRN2 NeuronCores.

Sharding: data-parallel over the batch dim B=8, one batch per core.
Per-core device algorithm (direction 1 shown; direction 2 swaps x<->y):
  The argmin_m ||x_n - y_m||^2 equals argmax_m s[n,m], s = 2 x.y - |y_m|^2,
  computed by the PE as a 4-dim augmented matmul:
     lhsT = [x0,x1,x2,1] (4 x 128-chunk), rhs = [2y0,2y1,2y2,-|y|^2] (4 x M).
  Per 128-row chunk, matmuls fill PSUM; the DVE tensor_tensor_scan (running
  max) produces r with the row max g at r[:, -1]; the Scalar engine computes
  Sign(g - r) with accum_out, which counts positions strictly before the
  first attainment of g — the exact argmax index with first-index
  tie-breaking (matching jnp.argmin).  An indirect DMA gathers the nearest
  neighbours, and a small epilogue accumulates sum_c |x - nn|^5 per row.
  Host combines per-core partial sums: mean_b((S1^.2 + S2^.2)).
"""

import numpy as np

import concourse.bass as bass
import concourse.bacc as bacc
import concourse.mybir as mybir
from concourse import bass_utils
from concourse.tile import TileContext

F32 = mybir.dt.float32
AF = mybir.ActivationFunctionType
ALU = mybir.AluOpType

B = 8
N_FULL = 4096
HALF_FULL = 2048
P = 128
NEG_BIG = -3.0e38
SIGN_ZERO_IS_ZERO = True


def _build_nc(N=N_FULL, HALF=HALF_FULL, num_devices=B):
    NCH = N // P
    MMFD = min(512, HALF)
    NH = N // HALF

    nc = bacc.Bacc("TRN2", target_bir_lowering=False,
                   num_devices=num_devices)

    # augs columns: [augx1 | augy1 | augy2 | augx2], each N wide.
    augs = nc.dram_tensor("augs", [4, 4 * N], F32, kind="ExternalInput").ap()
    xr = nc.dram_tensor("xr", [N, 3], F32, kind="ExternalInput").ap()
    yr = nc.dram_tensor("yr", [N, 3], F32, kind="ExternalInput").ap()
    out_s = nc.dram_tensor("out_s", [P, 2], F32, kind="ExternalOutput").ap()

    with TileContext(nc) as tc:
        with (
            tc.tile_pool(name="const", bufs=1) as const_pool,
            tc.tile_pool(name="r", bufs=2) as r_pool,
            tc.tile_pool(name="sgn", bufs=2) as sgn_pool,
            tc.tile_pool(name="idx", bufs=1) as idx_pool,
            tc.tile_pool(name="epi", bufs=1) as epi_pool,
            tc.tile_pool(name="psum", bufs=2, space="PSUM") as psum_pool,
        ):
            augs_sb = const_pool.tile([4, 4 * N], F32, tag="augs")
            nc.sync.dma_start(augs_sb[:], augs)

            def aug(i):  # [4, N] slice of the packed aug tile
                return augs_sb[:, i * N:(i + 1) * N]

            dummy = const_pool.tile([P, 1], F32, tag="dummy")
            nc.vector.memset(dummy[:], 0.0)

            idx_f = {1: idx_pool.tile([P, NCH], F32, tag="idxf1", name="idxf1"),
                     2: idx_pool.tile([P, NCH], F32, tag="idxf2", name="idxf2")}
            idx_i = {1: idx_pool.tile([P, NCH], mybir.dt.int32, tag="idxi1",
                                      name="idxi1"),
                     2: idx_pool.tile([P, NCH], mybir.dt.int32, tag="idxi2",
                                      name="idxi2")}
            # flat [P, NCH*3]: indirect-DMA dest slices must be a single
            # contiguous free dim (3D sliced dest APs are broken on HW)
            nn_t = {1: epi_pool.tile([P, NCH * 3], F32, tag="nn1", name="nn1"),
                    2: epi_pool.tile([P, NCH * 3], F32, tag="nn2", name="nn2")}

            for dirn in (1, 2):
                lhsT_all = aug(0) if dirn == 1 else aug(2)
                rhs_all = aug(1) if dirn == 1 else aug(3)
                gsrc = yr if dirn == 1 else xr
                for c in range(NCH):
                    r = r_pool.tile([P, N], F32, tag="r")
                    for h in range(NH):
                        ps = psum_pool.tile([P, HALF], F32, tag="ps",
                                            space="PSUM")
                        for k in range(HALF // MMFD):
                            m0 = h * HALF + k * MMFD
                            nc.tensor.matmul(
                                ps[:, k * MMFD:(k + 1) * MMFD],
                                lhsT=lhsT_all[:, c * P:(c + 1) * P],
                                rhs=rhs_all[:, m0:m0 + MMFD],
                                start=True, stop=True,
                            )
                        nc.vector.tensor_tensor_scan(
                            out=r[:, h * HALF:(h + 1) * HALF],
                            data0=ps[:],
                            data1=dummy[:, 0:1].to_broadcast([P, HALF]),
                            initial=(NEG_BIG if h == 0
                                     else r[:, h * HALF - 1:h * HALF]),
                            op0=ALU.max,
                            op1=ALU.bypass,
                        )
                    sgn = sgn_pool.tile([P, N], mybir.dt.bfloat16, tag="sgn")
                    if SIGN_ZERO_IS_ZERO:
                        # idx = sum_m Sign(g - r[m])   (Sign(0) == 0)
                        nc.scalar.activation(
                            out=sgn[:], in_=r[:, :],
                            func=AF.Sign,
                            bias=r[:, N - 1:N],
                            scale=-1.0,
                            accum_out=idx_f[dirn][:, c:c + 1],
                        )
                    else:
                        # count' = sum_m Sign(r[m] - g); idx = (N - count')/2
                        neg_g = epi_pool.tile([P, 1], F32, tag="negg")
                        nc.scalar.activation(
                            out=neg_g[:], in_=r[:, N - 1:N],
                            func=AF.Copy, bias=0.0, scale=-1.0,
                        )
                        nc.scalar.activation(
                            out=sgn[:], in_=r[:, :],
                            func=AF.Sign,
                            bias=neg_g[:],
                            scale=1.0,
                            accum_out=idx_f[dirn][:, c:c + 1],
                        )
                    # per-chunk index cast + NN gather (single offset per
                    # partition — multi-offset indirect DMA is broken on HW)
                    nc.vector.tensor_copy(idx_i[dirn][:, c:c + 1],
                                          idx_f[dirn][:, c:c + 1])
                    nc.gpsimd.indirect_dma_start(
                        out=nn_t[dirn][:, c * 3:(c + 1) * 3],
                        out_offset=None,
                        in_=gsrc,
                        in_offset=bass.IndirectOffsetOnAxis(
                            ap=idx_i[dirn][:, c:c + 1], axis=0),
                    )

            partials = epi_pool.tile([P, 2], F32, tag="partials")
            for dirn in (1, 2):
                own = xr if dirn == 1 else yr
                nn = nn_t[dirn]

                ow = epi_pool.tile([P, NCH, 3], F32, tag=f"ow{dirn}",
                                   name=f"ow{dirn}")
                nc.sync.dma_start(
                    ow[:], own.rearrange("(c p) d -> p c d", p=P))

                F = NCH * 3
                nnf = nn[:]
                owf = ow[:].rearrange("p a b -> p (a b)")
                diff = epi_pool.tile([P, F], F32, tag=f"diff{dirn}",
                                     name=f"diff{dirn}")
                nc.vector.tensor_sub(diff[:], owf, nnf)
                ad = epi_pool.tile([P, F], F32, tag=f"ad{dirn}",
                                   name=f"ad{dirn}")
                nc.scalar.activation(out=ad[:], in_=diff[:], func=AF.Abs,
                                     bias=0.0, scale=1.0)
                sq = epi_pool.tile([P, F], F32, tag=f"sq{dirn}",
                                   name=f"sq{dirn}")
                nc.scalar.activation(out=sq[:], in_=ad[:], func=AF.Square,
                                     bias=0.0, scale=1.0)
                q4 = epi_pool.tile([P, F], F32, tag=f"q4{dirn}",
                                   name=f"q4{dirn}")
                nc.scalar.activation(out=q4[:], in_=sq[:], func=AF.Square,
                                     bias=0.0, scale=1.0)
                p5 = epi_pool.tile([P, F], F32, tag=f"p5{dirn}",
                                   name=f"p5{dirn}")
                nc.vector.tensor_mul(p5[:], q4[:], ad[:])
                nc.vector.reduce_sum(partials[:, dirn - 1:dirn], p5[:],
                                     axis=mybir.AxisListType.X)

            nc.sync.dma_start(out_s, partials[:])

    nc.compile()
    return nc


def _host_prep(xb, yb):
    xb = np.ascontiguousarray(xb, dtype=np.float32)
    yb = np.ascontiguousarray(yb, dtype=np.float32)
    n = xb.shape[0]
    ones = np.ones((n,), np.float32)
    augs = np.empty((4, 4 * n), np.float32)
    augs[:, 0 * n:1 * n] = np.stack([xb[:, 0], xb[:, 1], xb[:, 2], ones])
    augs[:, 1 * n:2 * n] = np.stack(
        [2 * yb[:, 0], 2 * yb[:, 1], 2 * yb[:, 2], -(yb * yb).sum(-1)])
    augs[:, 2 * n:3 * n] = np.stack([yb[:, 0], yb[:, 1], yb[:, 2], ones])
    augs[:, 3 * n:4 * n] = np.stack(
        [2 * xb[:, 0], 2 * xb[:, 1], 2 * xb[:, 2], -(xb * xb).sum(-1)])
    return {"augs": augs, "xr": xb, "yr": yb}


_NC = None


def _get_nc():
    global _NC
    if _NC is None:
        _NC = _build_nc()
    return _NC


def run_on_hw(x, y, **spmd_kwargs):
    """Run the SPMD kernel; returns (per-core out arrays, BassKernelResults)."""
    x = np.asarray(x, dtype=np.float32)
    y = np.asarray(y, dtype=np.float32)
    assert x.shape == (B, N_FULL, 3) and y.shape == (B, N_FULL, 3)
    nc = _get_nc()
    in_maps = [_host_prep(x[b], y[b]) for b in range(B)]
    res = bass_utils.run_bass_kernel_spmd(
        nc, in_maps, core_ids=list(range(B)), **spmd_kwargs)
    outs = [res.results[b]["out_s"] for b in range(B)]
    return outs, res


def kernel(x, y):
    outs, _ = run_on_hw(x, y)
    vals = []
    for o in outs:
        s = np.asarray(o, dtype=np.float64).sum(axis=0)
        vals.append(s[0] ** 0.2 + s[1] ** 0.2)
    return np.float32(np.mean(vals))


# revision 11
# speedup vs baseline: 2.3293x; 2.3293x over previous
"""Chamfer p=5 loss (nn_ChamferLossP) — Bass kernel for 8x TRN2 NeuronCores.

Sharding: data-parallel over the batch dim B=8, one batch per core; host
combines the per-core partial sums (the final "mean all-reduce").

Per-core device algorithm (direction 1 shown; direction 2 swaps x<->y):

  argmin_m ||x_n - y_m||^2  ==  argmax_m s[n,m],  s = 2 x.y - |y_m|^2.

  The PE materialises s in PSUM tiles [128n x 512m] with a single bf16
  matmul per tile: each fp32 factor is split into 3 bf16 limbs and the
  6 significant limb products per coordinate (plus 3 limbs of the -|y|^2
  term) form a 21-term contraction — fp32-accurate keys (~1e-7 rel) at
  bf16 speed (4x faster than the PE's multi-pass fp32 mode).

  Index extraction is two-level: the DVE reduces each PSUM tile to
  16-element group maxima (1 elem/cycle), then a short tensor_tensor_scan
  (running max, 2 cycles/elem but only N/16 elems) produces the prefix
  maxima r8 whose last column is the row max g.  The Scalar engine's
  Sign(g - r8) with accum_out counts groups strictly before the first
  attainment of g — the exact first-attainment group index (Sign(0)=0 on
  HW, probed).  One indirect DMA per 128-row chunk gathers that group's
  16 candidate points (48 contiguous floats); the epilogue recomputes the
  16 exact fp32 squared distances, picks the winner (first index on
  ties, matching jnp.argmin), and accumulates sum_c |x - nn|^5.
"""

import numpy as np
import ml_dtypes

import concourse.bass as bass
import concourse.bacc as bacc
import concourse.mybir as mybir
from concourse import bass_utils
from concourse.tile import TileContext

F32 = mybir.dt.float32
BF16 = mybir.dt.bfloat16
AF = mybir.ActivationFunctionType
ALU = mybir.AluOpType

B = 8
N_FULL = 4096
HALF_FULL = 2048
P = 128
R = 16              # argmin group size (candidates per gather)
KSPLIT = 21         # bf16 split-contraction terms
NEG_BIG = -3.0e38


def _build_nc(N=N_FULL, HALF=HALF_FULL, num_devices=B):
    NCH = N // P         # 128-row chunks per direction
    MMFD = min(512, HALF)
    NH = N // HALF       # psum tiles per chunk
    NG = N // R          # groups per row
    GH = HALF // R       # groups per psum tile

    nc = bacc.Bacc("TRN2", target_bir_lowering=False,
                   num_devices=num_devices)

    # augs columns: [x1_lhsT | y1_rhs | y2_lhsT | x2_rhs], each N wide, bf16.
    augs = nc.dram_tensor("augs", [KSPLIT, 4 * N], BF16,
                          kind="ExternalInput").ap()
    xr = nc.dram_tensor("xr", [N, 3], F32, kind="ExternalInput").ap()
    yr = nc.dram_tensor("yr", [N, 3], F32, kind="ExternalInput").ap()
    # consts row: [iota16 | iota16 + R]
    consts = nc.dram_tensor("consts", [P, 2 * R], F32,
                            kind="ExternalInput").ap()
    out_s = nc.dram_tensor("out_s", [P, 2], F32, kind="ExternalOutput").ap()

    with TileContext(nc) as tc:
        with (
            tc.tile_pool(name="const", bufs=1) as const_pool,
            tc.tile_pool(name="u", bufs=2) as u_pool,
            tc.tile_pool(name="r8", bufs=2) as r8_pool,
            tc.tile_pool(name="sgn", bufs=2) as sgn_pool,
            tc.tile_pool(name="idx", bufs=1) as idx_pool,
            tc.tile_pool(name="epi", bufs=1) as epi_pool,
            tc.tile_pool(name="psum", bufs=2, space="PSUM") as psum_pool,
        ):
            augs_sb = const_pool.tile([KSPLIT, 4 * N], BF16, tag="augs")
            nc.sync.dma_start(augs_sb[:], augs)

            def aug(i):
                return augs_sb[:, i * N:(i + 1) * N]

            consts_sb = const_pool.tile([P, 2 * R], F32, tag="consts")
            nc.sync.dma_start(consts_sb[:], consts)

            dummy = const_pool.tile([P, 1], F32, tag="dummy")
            nc.vector.memset(dummy[:], 0.0)

            idxg_f = {1: idx_pool.tile([P, NCH], F32, tag="ig1", name="ig1"),
                      2: idx_pool.tile([P, NCH], F32, tag="ig2", name="ig2")}
            idxg_i = {1: idx_pool.tile([P, NCH], mybir.dt.int32, tag="ii1",
                                       name="ii1"),
                      2: idx_pool.tile([P, NCH], mybir.dt.int32, tag="ii2",
                                       name="ii2")}
            # gathered candidate groups, flat [P, NCH * R * 3]
            cand = {1: epi_pool.tile([P, NCH * R * 3], F32, tag="cand1",
                                     name="cand1"),
                    2: epi_pool.tile([P, NCH * R * 3], F32, tag="cand2",
                                     name="cand2")}

            for dirn in (1, 2):
                lhsT_all = aug(0) if dirn == 1 else aug(2)
                rhs_all = aug(1) if dirn == 1 else aug(3)
                gsrc = yr if dirn == 1 else xr
                gsrc_g = gsrc.rearrange("(g k) d -> g (k d)", k=R)
                for c in range(NCH):
                    r8 = r8_pool.tile([P, NG], F32, tag="r8")
                    for h in range(NH):
                        ps = psum_pool.tile([P, HALF], F32, tag="ps",
                                            space="PSUM")
                        for k in range(HALF // MMFD):
                            m0 = h * HALF + k * MMFD
                            nc.tensor.matmul(
                                ps[:, k * MMFD:(k + 1) * MMFD],
                                lhsT=lhsT_all[:, c * P:(c + 1) * P],
                                rhs=rhs_all[:, m0:m0 + MMFD],
                                start=True, stop=True,
                            )
                        # group maxima (R-wide) of this tile, 1 elem/cyc
                        u = u_pool.tile([P, GH], F32, tag="u")
                        nc.vector.tensor_reduce(
                            out=u[:],
                            in_=ps[:].rearrange("p (g k) -> p g k", k=R),
                            axis=mybir.AxisListType.X,
                            op=ALU.max,
                        )
                        # prefix max over groups (2 cyc/elem, N/R elems)
                        nc.vector.tensor_tensor_scan(
                            out=r8[:, h * GH:(h + 1) * GH],
                            data0=u[:],
                            data1=dummy[:, 0:1].to_broadcast([P, GH]),
                            initial=(NEG_BIG if h == 0
                                     else r8[:, h * GH - 1:h * GH]),
                            op0=ALU.max,
                            op1=ALU.bypass,
                        )
                    # group index of first attainment of the row max
                    sgn = sgn_pool.tile([P, NG], BF16, tag="sgn")
                    nc.scalar.activation(
                        out=sgn[:], in_=r8[:, :],
                        func=AF.Sign,
                        bias=r8[:, NG - 1:NG],
                        scale=-1.0,
                        accum_out=idxg_f[dirn][:, c:c + 1],
                    )
                    nc.vector.tensor_copy(idxg_i[dirn][:, c:c + 1],
                                          idxg_f[dirn][:, c:c + 1])
                    # gather the 16-candidate group (48 contiguous floats)
                    nc.gpsimd.indirect_dma_start(
                        out=cand[dirn][:, c * R * 3:(c + 1) * R * 3],
                        out_offset=None,
                        in_=gsrc_g,
                        in_offset=bass.IndirectOffsetOnAxis(
                            ap=idxg_i[dirn][:, c:c + 1], axis=0),
                    )

            # ---- epilogue: exact within-group argmin + sum |diff|^5 ----
            partials = epi_pool.tile([P, 2], F32, tag="partials")
            FC = NCH * R * 3   # candidate floats per partition
            FK = NCH * R       # candidates per partition
            for dirn in (1, 2):
                own = xr if dirn == 1 else yr
                cd = cand[dirn]

                ow = epi_pool.tile([P, NCH, 3], F32, tag=f"ow{dirn}",
                                   name=f"ow{dirn}")
                nc.sync.dma_start(
                    ow[:], own.rearrange("(c p) d -> p c d", p=P))
                # own broadcast over the R candidates: [P, NCH, R, 3]
                owb = bass.AP(ow[:].tensor, ow[:].offset,
                              [ow[:].ap[0], [3, NCH], [0, R], [1, 3]])

                diff = epi_pool.tile([P, FC], F32, tag=f"df{dirn}",
                                     name=f"df{dirn}")
                nc.vector.tensor_sub(
                    diff[:].rearrange("p (c k d) -> p c k d", k=R, d=3),
                    owb, cd[:].rearrange("p (c k d) -> p c k d", k=R, d=3))
                ad = epi_pool.tile([P, FC], F32, tag=f"ab{dirn}",
                                   name=f"ab{dirn}")
                nc.scalar.activation(out=ad[:], in_=diff[:], func=AF.Abs,
                                     bias=0.0, scale=1.0)
                sq = epi_pool.tile([P, FC], F32, tag=f"sq{dirn}",
                                   name=f"sq{dirn}")
                nc.scalar.activation(out=sq[:], in_=ad[:], func=AF.Square,
                                     bias=0.0, scale=1.0)
                # squared L2 distance per candidate [P, NCH*R]
                dd = epi_pool.tile([P, FK], F32, tag=f"dd{dirn}",
                                   name=f"dd{dirn}")
                nc.vector.tensor_reduce(
                    out=dd[:], in_=sq[:].rearrange("p (k d) -> p k d", d=3),
                    axis=mybir.AxisListType.X, op=ALU.add)
                # min distance per row [P, NCH]
                dmin = epi_pool.tile([P, NCH], F32, tag=f"dm{dirn}",
                                     name=f"dm{dirn}")
                nc.vector.tensor_reduce(
                    out=dmin[:], in_=dd[:].rearrange("p (c k) -> p c k", k=R),
                    axis=mybir.AxisListType.X, op=ALU.min)
                dminb = bass.AP(dmin[:].tensor, dmin[:].offset,
                                [dmin[:].ap[0], [1, NCH], [0, R]])
                mask = epi_pool.tile([P, FK], F32, tag=f"mk{dirn}",
                                     name=f"mk{dirn}")
                nc.vector.tensor_tensor(
                    out=mask[:].rearrange("p (c k) -> p c k", k=R),
                    in0=dd[:].rearrange("p (c k) -> p c k", k=R),
                    in1=dminb, op=ALU.is_le)
                # first-attaining candidate: k* = min_k (iota_k + 16*(1-mask))
                iotap = bass.AP(consts_sb[:].tensor, consts_sb[:].offset + R,
                               [consts_sb[:].ap[0], [0, NCH], [1, R]])
                tk = epi_pool.tile([P, FK], F32, tag=f"tk{dirn}",
                                   name=f"tk{dirn}")
                nc.vector.scalar_tensor_tensor(
                    out=tk[:].rearrange("p (c k) -> p c k", k=R),
                    in0=mask[:].rearrange("p (c k) -> p c k", k=R),
                    scalar=-float(R), op0=ALU.mult,
                    in1=iotap, op1=ALU.add)
                kstar = epi_pool.tile([P, NCH], F32, tag=f"ks{dirn}",
                                      name=f"ks{dirn}")
                nc.vector.tensor_reduce(
                    out=kstar[:], in_=tk[:].rearrange("p (c k) -> p c k", k=R),
                    axis=mybir.AxisListType.X, op=ALU.min)
                # kstar is iota[k*]+R-R... note tk = iota+R-16*mask so the
                # masked entries are iota exactly; min = k*.
                ksb = bass.AP(kstar[:].tensor, kstar[:].offset,
                              [kstar[:].ap[0], [1, NCH], [0, R]])
                onehot = epi_pool.tile([P, FK], F32, tag=f"oh{dirn}",
                                       name=f"oh{dirn}")
                nc.vector.tensor_tensor(
                    out=onehot[:].rearrange("p (c k) -> p c k", k=R),
                    in0=bass.AP(consts_sb[:].tensor, consts_sb[:].offset,
                                [consts_sb[:].ap[0], [0, NCH], [1, R]]),
                    in1=ksb, op=ALU.is_equal)
                # p5 per candidate, then select the winner
                q4 = epi_pool.tile([P, FC], F32, tag=f"q4{dirn}",
                                   name=f"q4{dirn}")
                nc.scalar.activation(out=q4[:], in_=sq[:], func=AF.Square,
                                     bias=0.0, scale=1.0)
                p5e = epi_pool.tile([P, FC], F32, tag=f"p5{dirn}",
                                    name=f"p5{dirn}")
                nc.vector.tensor_mul(p5e[:], q4[:], ad[:])
                p5k = epi_pool.tile([P, FK], F32, tag=f"pk{dirn}",
                                    name=f"pk{dirn}")
                nc.vector.tensor_reduce(
                    out=p5k[:], in_=p5e[:].rearrange("p (k d) -> p k d", d=3),
                    axis=mybir.AxisListType.X, op=ALU.add)
                psel = epi_pool.tile([P, FK], F32, tag=f"pl{dirn}",
                                     name=f"pl{dirn}")
                nc.vector.tensor_mul(psel[:], p5k[:], onehot[:])
                nc.vector.reduce_sum(partials[:, dirn - 1:dirn], psel[:],
                                     axis=mybir.AxisListType.X)

            nc.sync.dma_start(out_s, partials[:])

    nc.compile()
    return nc


def _to_bf16(a):
    return a.astype(ml_dtypes.bfloat16)


def _split3(a):
    a = np.asarray(a, np.float32)
    h = _to_bf16(a)
    m = _to_bf16(a - h.astype(np.float32))
    l = _to_bf16(a - h.astype(np.float32) - m.astype(np.float32))
    return h, m, l


def _host_prep(xb, yb):
    xb = np.ascontiguousarray(xb, dtype=np.float32)
    yb = np.ascontiguousarray(yb, dtype=np.float32)
    n = xb.shape[0]
    ones = np.ones((n,), np.float32)

    def build(sta, mov, key_sq):
        """bf16 split terms for s = sum_c sta_c * (2 mov_c) - |mov|^2
        as seen with `sta` stationary; key_sq = -(|mov|^2)."""
        ta, tb = [], []
        for c in range(3):
            a, b = _split3(sta[:, c]), _split3(2.0 * mov[:, c])
            for i, j in ((0, 0), (0, 1), (0, 2), (1, 0), (1, 1), (2, 0)):
                ta.append(a[i])
                tb.append(b[j])
        sh, sm, sl = _split3(key_sq)
        ob = _to_bf16(ones)
        for s in (sh, sm, sl):
            ta.append(ob)
            tb.append(s)
        A = np.stack(ta).astype(ml_dtypes.bfloat16)
        Bm = np.stack(tb).astype(ml_dtypes.bfloat16)
        return A, Bm

    y2 = -(yb * yb).sum(-1)
    x2 = -(xb * xb).sum(-1)
    A1, B1 = build(xb, yb, y2)   # dir 1: lhsT = x terms, rhs = y terms
    A2, B2 = build(yb, xb, x2)   # dir 2: lhsT = y terms, rhs = x terms

    augs = np.empty((KSPLIT, 4 * n), ml_dtypes.bfloat16)
    augs[:, 0 * n:1 * n] = A1
    augs[:, 1 * n:2 * n] = B1
    augs[:, 2 * n:3 * n] = A2
    augs[:, 3 * n:4 * n] = B2

    iota = np.arange(R, dtype=np.float32)
    consts = np.tile(np.concatenate([iota, iota + R])[None, :], (P, 1))
    return {"augs": augs, "xr": xb, "yr": yb,
            "consts": np.ascontiguousarray(consts, np.float32)}


_NC = None


def _get_nc():
    global _NC
    if _NC is None:
        _NC = _build_nc()
    return _NC


def run_on_hw(x, y, **spmd_kwargs):
    """Run the SPMD kernel; returns (per-core out arrays, BassKernelResults)."""
    x = np.asarray(x, dtype=np.float32)
    y = np.asarray(y, dtype=np.float32)
    assert x.shape == (B, N_FULL, 3) and y.shape == (B, N_FULL, 3)
    nc = _get_nc()
    in_maps = [_host_prep(x[b], y[b]) for b in range(B)]
    res = bass_utils.run_bass_kernel_spmd(
        nc, in_maps, core_ids=list(range(B)), **spmd_kwargs)
    outs = [res.results[b]["out_s"] for b in range(B)]
    return outs, res


def kernel(x, y):
    outs, _ = run_on_hw(x, y)
    vals = []
    for o in outs:
        s = np.asarray(o, dtype=np.float64).sum(axis=0)
        vals.append(s[0] ** 0.2 + s[1] ** 0.2)
    return np.float32(np.mean(vals))


# revision 12
# speedup vs baseline: 2.4080x; 1.0338x over previous
"""Chamfer p=5 loss (nn_ChamferLossP) — Bass kernel for 8x TRN2 NeuronCores.

Sharding: data-parallel over the batch dim B=8, one batch per core; host
combines the per-core partial sums (the final "mean all-reduce").

Per-core device algorithm (direction 1 shown; direction 2 swaps x<->y):

  argmin_m ||x_n - y_m||^2  ==  argmax_m s[n,m],  s = 2 x.y - |y_m|^2.

  The PE materialises s in PSUM tiles [128n x 512m] with a single bf16
  matmul per tile: each fp32 factor is split into 3 bf16 limbs and the
  6 significant limb products per coordinate (plus 3 limbs of the -|y|^2
  term) form a 21-term contraction — fp32-accurate keys (~1e-7 rel) at
  bf16 speed (4x faster than the PE's multi-pass fp32 mode).

  Index extraction is two-level: the DVE reduces each PSUM tile to
  16-element group maxima (1 elem/cycle), then a short tensor_tensor_scan
  (running max, 2 cycles/elem but only N/16 elems) produces the prefix
  maxima r8 whose last column is the row max g.  The Scalar engine's
  Sign(g - r8) with accum_out counts groups strictly before the first
  attainment of g — the exact first-attainment group index (Sign(0)=0 on
  HW, probed).  One indirect DMA per 128-row chunk gathers that group's
  16 candidate points (48 contiguous floats); the epilogue recomputes the
  16 exact fp32 squared distances, picks the winner (first index on
  ties, matching jnp.argmin), and accumulates sum_c |x - nn|^5.
"""

import numpy as np
import ml_dtypes

import concourse.bass as bass
import concourse.bacc as bacc
import concourse.mybir as mybir
from concourse import bass_utils
from concourse.tile import TileContext

F32 = mybir.dt.float32
BF16 = mybir.dt.bfloat16
AF = mybir.ActivationFunctionType
ALU = mybir.AluOpType

B = 8
N_FULL = 4096
HALF_FULL = 2048
P = 128
R = 16              # argmin group size (candidates per gather)
KSPLIT = 21         # bf16 split-contraction terms
NEG_BIG = -3.0e38


def _build_nc(N=N_FULL, HALF=HALF_FULL, num_devices=B):
    NCH = N // P         # 128-row chunks per direction
    MMFD = min(512, HALF)
    NH = N // HALF       # psum tiles per chunk
    NG = N // R          # groups per row
    GH = HALF // R       # groups per psum tile

    nc = bacc.Bacc("TRN2", target_bir_lowering=False,
                   num_devices=num_devices)

    # augs columns: [x1_lhsT | y1_rhs | y2_lhsT | x2_rhs], each N wide, bf16.
    augs = nc.dram_tensor("augs", [KSPLIT, 4 * N], BF16,
                          kind="ExternalInput").ap()
    xr = nc.dram_tensor("xr", [N, 3], F32, kind="ExternalInput").ap()
    yr = nc.dram_tensor("yr", [N, 3], F32, kind="ExternalInput").ap()
    # consts row: [iota16 | iota16 + R]
    consts = nc.dram_tensor("consts", [P, 2 * R], F32,
                            kind="ExternalInput").ap()
    out_s = nc.dram_tensor("out_s", [P, 2], F32, kind="ExternalOutput").ap()

    with TileContext(nc) as tc:
        with (
            tc.tile_pool(name="const", bufs=1) as const_pool,
            tc.tile_pool(name="u", bufs=3) as u_pool,
            tc.tile_pool(name="r8", bufs=3) as r8_pool,
            tc.tile_pool(name="sgn", bufs=3) as sgn_pool,
            tc.tile_pool(name="idx", bufs=1) as idx_pool,
            tc.tile_pool(name="epi", bufs=1) as epi_pool,
            tc.tile_pool(name="psum", bufs=2, space="PSUM") as psum_pool,
        ):
            augs_sb = const_pool.tile([KSPLIT, 4 * N], BF16, tag="augs")
            nc.sync.dma_start(augs_sb[:], augs)

            def aug(i):
                return augs_sb[:, i * N:(i + 1) * N]

            consts_sb = const_pool.tile([P, 2 * R], F32, tag="consts")
            nc.sync.dma_start(consts_sb[:], consts)

            dummy = const_pool.tile([P, 1], F32, tag="dummy")
            nc.vector.memset(dummy[:], 0.0)

            idxg_f = {1: idx_pool.tile([P, NCH], F32, tag="ig1", name="ig1"),
                      2: idx_pool.tile([P, NCH], F32, tag="ig2", name="ig2")}
            idxg_i = {1: idx_pool.tile([P, NCH], mybir.dt.int32, tag="ii1",
                                       name="ii1"),
                      2: idx_pool.tile([P, NCH], mybir.dt.int32, tag="ii2",
                                       name="ii2")}
            # gathered candidate groups, flat [P, NCH * R * 3]
            cand = {1: epi_pool.tile([P, NCH * R * 3], F32, tag="cand1",
                                     name="cand1"),
                    2: epi_pool.tile([P, NCH * R * 3], F32, tag="cand2",
                                     name="cand2")}

            for dirn in (1, 2):
                lhsT_all = aug(0) if dirn == 1 else aug(2)
                rhs_all = aug(1) if dirn == 1 else aug(3)
                gsrc = yr if dirn == 1 else xr
                gsrc_g = gsrc.rearrange("(g k) d -> g (k d)", k=R)
                for c in range(NCH):
                    r8 = r8_pool.tile([P, NG], F32, tag="r8")
                    for h in range(NH):
                        ps = psum_pool.tile([P, HALF], F32, tag="ps",
                                            space="PSUM")
                        for k in range(HALF // MMFD):
                            m0 = h * HALF + k * MMFD
                            nc.tensor.matmul(
                                ps[:, k * MMFD:(k + 1) * MMFD],
                                lhsT=lhsT_all[:, c * P:(c + 1) * P],
                                rhs=rhs_all[:, m0:m0 + MMFD],
                                start=True, stop=True,
                            )
                        # group maxima (R-wide) of this tile, 1 elem/cyc
                        u = u_pool.tile([P, GH], F32, tag="u")
                        nc.vector.tensor_reduce(
                            out=u[:],
                            in_=ps[:].rearrange("p (g k) -> p g k", k=R),
                            axis=mybir.AxisListType.X,
                            op=ALU.max,
                        )
                        # prefix max over groups (2 cyc/elem, N/R elems)
                        nc.vector.tensor_tensor_scan(
                            out=r8[:, h * GH:(h + 1) * GH],
                            data0=u[:],
                            data1=dummy[:, 0:1].to_broadcast([P, GH]),
                            initial=(NEG_BIG if h == 0
                                     else r8[:, h * GH - 1:h * GH]),
                            op0=ALU.max,
                            op1=ALU.bypass,
                        )
                    # group index of first attainment of the row max
                    sgn = sgn_pool.tile([P, NG], BF16, tag="sgn")
                    nc.scalar.activation(
                        out=sgn[:], in_=r8[:, :],
                        func=AF.Sign,
                        bias=r8[:, NG - 1:NG],
                        scale=-1.0,
                        accum_out=idxg_f[dirn][:, c:c + 1],
                    )
                    nc.vector.tensor_copy(idxg_i[dirn][:, c:c + 1],
                                          idxg_f[dirn][:, c:c + 1])
                    # gather the 16-candidate group (48 contiguous floats)
                    nc.gpsimd.indirect_dma_start(
                        out=cand[dirn][:, c * R * 3:(c + 1) * R * 3],
                        out_offset=None,
                        in_=gsrc_g,
                        in_offset=bass.IndirectOffsetOnAxis(
                            ap=idxg_i[dirn][:, c:c + 1], axis=0),
                    )

            # ---- epilogue: exact within-group argmin + sum |diff|^5 ----
            partials = epi_pool.tile([P, 2], F32, tag="partials")
            FC = NCH * R * 3   # candidate floats per partition
            FK = NCH * R       # candidates per partition
            for dirn in (1, 2):
                own = xr if dirn == 1 else yr
                cd = cand[dirn]

                ow = epi_pool.tile([P, NCH, 3], F32, tag=f"ow{dirn}",
                                   name=f"ow{dirn}")
                nc.sync.dma_start(
                    ow[:], own.rearrange("(c p) d -> p c d", p=P))
                # own broadcast over the R candidates: [P, NCH, R, 3]
                owb = bass.AP(ow[:].tensor, ow[:].offset,
                              [ow[:].ap[0], [3, NCH], [0, R], [1, 3]])

                diff = epi_pool.tile([P, FC], F32, tag=f"df{dirn}",
                                     name=f"df{dirn}")
                nc.vector.tensor_sub(
                    diff[:].rearrange("p (c k d) -> p c k d", k=R, d=3),
                    owb, cd[:].rearrange("p (c k d) -> p c k d", k=R, d=3))
                ad = epi_pool.tile([P, FC], F32, tag=f"ab{dirn}",
                                   name=f"ab{dirn}")
                nc.scalar.activation(out=ad[:], in_=diff[:], func=AF.Abs,
                                     bias=0.0, scale=1.0)
                sq = epi_pool.tile([P, FC], F32, tag=f"sq{dirn}",
                                   name=f"sq{dirn}")
                nc.scalar.activation(out=sq[:], in_=ad[:], func=AF.Square,
                                     bias=0.0, scale=1.0)
                # squared L2 distance per candidate [P, NCH*R]
                dd = epi_pool.tile([P, FK], F32, tag=f"dd{dirn}",
                                   name=f"dd{dirn}")
                nc.vector.tensor_reduce(
                    out=dd[:], in_=sq[:].rearrange("p (k d) -> p k d", d=3),
                    axis=mybir.AxisListType.X, op=ALU.add)
                # min distance per row [P, NCH]
                dmin = epi_pool.tile([P, NCH], F32, tag=f"dm{dirn}",
                                     name=f"dm{dirn}")
                nc.vector.tensor_reduce(
                    out=dmin[:], in_=dd[:].rearrange("p (c k) -> p c k", k=R),
                    axis=mybir.AxisListType.X, op=ALU.min)
                dminb = bass.AP(dmin[:].tensor, dmin[:].offset,
                                [dmin[:].ap[0], [1, NCH], [0, R]])
                mask = epi_pool.tile([P, FK], F32, tag=f"mk{dirn}",
                                     name=f"mk{dirn}")
                nc.vector.tensor_tensor(
                    out=mask[:].rearrange("p (c k) -> p c k", k=R),
                    in0=dd[:].rearrange("p (c k) -> p c k", k=R),
                    in1=dminb, op=ALU.is_le)
                # first-attaining candidate: k* = min_k (iota_k + 16*(1-mask))
                iotap = bass.AP(consts_sb[:].tensor, consts_sb[:].offset + R,
                               [consts_sb[:].ap[0], [0, NCH], [1, R]])
                tk = epi_pool.tile([P, FK], F32, tag=f"tk{dirn}",
                                   name=f"tk{dirn}")
                nc.vector.scalar_tensor_tensor(
                    out=tk[:].rearrange("p (c k) -> p c k", k=R),
                    in0=mask[:].rearrange("p (c k) -> p c k", k=R),
                    scalar=-float(R), op0=ALU.mult,
                    in1=iotap, op1=ALU.add)
                kstar = epi_pool.tile([P, NCH], F32, tag=f"ks{dirn}",
                                      name=f"ks{dirn}")
                nc.vector.tensor_reduce(
                    out=kstar[:], in_=tk[:].rearrange("p (c k) -> p c k", k=R),
                    axis=mybir.AxisListType.X, op=ALU.min)
                # kstar is iota[k*]+R-R... note tk = iota+R-16*mask so the
                # masked entries are iota exactly; min = k*.
                ksb = bass.AP(kstar[:].tensor, kstar[:].offset,
                              [kstar[:].ap[0], [1, NCH], [0, R]])
                onehot = epi_pool.tile([P, FK], F32, tag=f"oh{dirn}",
                                       name=f"oh{dirn}")
                nc.vector.tensor_tensor(
                    out=onehot[:].rearrange("p (c k) -> p c k", k=R),
                    in0=bass.AP(consts_sb[:].tensor, consts_sb[:].offset,
                                [consts_sb[:].ap[0], [0, NCH], [1, R]]),
                    in1=ksb, op=ALU.is_equal)
                # p5 per candidate, then select the winner
                q4 = epi_pool.tile([P, FC], F32, tag=f"q4{dirn}",
                                   name=f"q4{dirn}")
                nc.scalar.activation(out=q4[:], in_=sq[:], func=AF.Square,
                                     bias=0.0, scale=1.0)
                p5e = epi_pool.tile([P, FC], F32, tag=f"p5{dirn}",
                                    name=f"p5{dirn}")
                nc.vector.tensor_mul(p5e[:], q4[:], ad[:])
                p5k = epi_pool.tile([P, FK], F32, tag=f"pk{dirn}",
                                    name=f"pk{dirn}")
                nc.vector.tensor_reduce(
                    out=p5k[:], in_=p5e[:].rearrange("p (k d) -> p k d", d=3),
                    axis=mybir.AxisListType.X, op=ALU.add)
                psel = epi_pool.tile([P, FK], F32, tag=f"pl{dirn}",
                                     name=f"pl{dirn}")
                nc.vector.tensor_mul(psel[:], p5k[:], onehot[:])
                nc.vector.reduce_sum(partials[:, dirn - 1:dirn], psel[:],
                                     axis=mybir.AxisListType.X)

            nc.sync.dma_start(out_s, partials[:])

    nc.compile()
    return nc


def _to_bf16(a):
    return a.astype(ml_dtypes.bfloat16)


def _split3(a):
    a = np.asarray(a, np.float32)
    h = _to_bf16(a)
    m = _to_bf16(a - h.astype(np.float32))
    l = _to_bf16(a - h.astype(np.float32) - m.astype(np.float32))
    return h, m, l


def _host_prep(xb, yb):
    xb = np.ascontiguousarray(xb, dtype=np.float32)
    yb = np.ascontiguousarray(yb, dtype=np.float32)
    n = xb.shape[0]
    ones = np.ones((n,), np.float32)

    def build(sta, mov, key_sq):
        """bf16 split terms for s = sum_c sta_c * (2 mov_c) - |mov|^2
        as seen with `sta` stationary; key_sq = -(|mov|^2)."""
        ta, tb = [], []
        for c in range(3):
            a, b = _split3(sta[:, c]), _split3(2.0 * mov[:, c])
            for i, j in ((0, 0), (0, 1), (0, 2), (1, 0), (1, 1), (2, 0)):
                ta.append(a[i])
                tb.append(b[j])
        sh, sm, sl = _split3(key_sq)
        ob = _to_bf16(ones)
        for s in (sh, sm, sl):
            ta.append(ob)
            tb.append(s)
        A = np.stack(ta).astype(ml_dtypes.bfloat16)
        Bm = np.stack(tb).astype(ml_dtypes.bfloat16)
        return A, Bm

    y2 = -(yb * yb).sum(-1)
    x2 = -(xb * xb).sum(-1)
    A1, B1 = build(xb, yb, y2)   # dir 1: lhsT = x terms, rhs = y terms
    A2, B2 = build(yb, xb, x2)   # dir 2: lhsT = y terms, rhs = x terms

    augs = np.empty((KSPLIT, 4 * n), ml_dtypes.bfloat16)
    augs[:, 0 * n:1 * n] = A1
    augs[:, 1 * n:2 * n] = B1
    augs[:, 2 * n:3 * n] = A2
    augs[:, 3 * n:4 * n] = B2

    iota = np.arange(R, dtype=np.float32)
    consts = np.tile(np.concatenate([iota, iota + R])[None, :], (P, 1))
    return {"augs": augs, "xr": xb, "yr": yb,
            "consts": np.ascontiguousarray(consts, np.float32)}


_NC = None


def _get_nc():
    global _NC
    if _NC is None:
        _NC = _build_nc()
    return _NC


def run_on_hw(x, y, **spmd_kwargs):
    """Run the SPMD kernel; returns (per-core out arrays, BassKernelResults)."""
    x = np.asarray(x, dtype=np.float32)
    y = np.asarray(y, dtype=np.float32)
    assert x.shape == (B, N_FULL, 3) and y.shape == (B, N_FULL, 3)
    nc = _get_nc()
    in_maps = [_host_prep(x[b], y[b]) for b in range(B)]
    res = bass_utils.run_bass_kernel_spmd(
        nc, in_maps, core_ids=list(range(B)), **spmd_kwargs)
    outs = [res.results[b]["out_s"] for b in range(B)]
    return outs, res


def kernel(x, y):
    outs, _ = run_on_hw(x, y)
    vals = []
    for o in outs:
        s = np.asarray(o, dtype=np.float64).sum(axis=0)
        vals.append(s[0] ** 0.2 + s[1] ** 0.2)
    return np.float32(np.mean(vals))


# revision 15
# speedup vs baseline: 2.4476x; 1.0165x over previous
"""Chamfer p=5 loss (nn_ChamferLossP) — Bass kernel for 8x TRN2 NeuronCores.

Sharding: data-parallel over the batch dim B=8, one batch per core; host
combines the per-core partial sums (the final "mean all-reduce").

Per-core device algorithm (direction 1 shown; direction 2 swaps x<->y):

  argmin_m ||x_n - y_m||^2  ==  argmax_m s[n,m],  s = 2 x.y - |y_m|^2.

  The PE materialises s in PSUM tiles [128n x 512m] with a single bf16
  matmul per tile: each fp32 factor is split into 3 bf16 limbs and the
  6 significant limb products per coordinate (plus 3 limbs of the -|y|^2
  term) form a 21-term contraction — fp32-accurate keys (~1e-7 rel) at
  bf16 speed (4x faster than the PE's multi-pass fp32 mode).

  Index extraction is two-level: the DVE reduces each PSUM tile to
  16-element group maxima (1 elem/cycle), then a short tensor_tensor_scan
  (running max, 2 cycles/elem but only N/16 elems) produces the prefix
  maxima r8 whose last column is the row max g.  The Scalar engine's
  Sign(g - r8) with accum_out counts groups strictly before the first
  attainment of g — the exact first-attainment group index (Sign(0)=0 on
  HW, probed).  One indirect DMA per 128-row chunk gathers that group's
  16 candidate points (48 contiguous floats); the epilogue recomputes the
  16 exact fp32 squared distances, picks the winner (first index on
  ties, matching jnp.argmin), and accumulates sum_c |x - nn|^5.
"""

import numpy as np
import ml_dtypes

import concourse.bass as bass
import concourse.bacc as bacc
import concourse.mybir as mybir
from concourse import bass_utils
from concourse.tile import TileContext

F32 = mybir.dt.float32
BF16 = mybir.dt.bfloat16
AF = mybir.ActivationFunctionType
ALU = mybir.AluOpType

B = 8
N_FULL = 4096
HALF_FULL = 2048
P = 128
R = 16              # argmin group size (candidates per gather)
KSPLIT = 21         # bf16 split-contraction terms
NEG_BIG = -3.0e38


def _build_nc(N=N_FULL, HALF=HALF_FULL, num_devices=B):
    NCH = N // P         # 128-row chunks per direction
    MMFD = min(512, HALF)
    NH = N // HALF       # psum tiles per chunk
    NG = N // R          # groups per row
    GH = HALF // R       # groups per psum tile

    nc = bacc.Bacc("TRN2", target_bir_lowering=False,
                   num_devices=num_devices)

    # augs columns: [x1_lhsT | y1_rhs | y2_lhsT | x2_rhs], each N wide, bf16.
    augs = nc.dram_tensor("augs", [KSPLIT, 4 * N], BF16,
                          kind="ExternalInput").ap()
    xr = nc.dram_tensor("xr", [N, 3], F32, kind="ExternalInput").ap()
    yr = nc.dram_tensor("yr", [N, 3], F32, kind="ExternalInput").ap()
    # consts row: [iota16 | iota16 + R]
    consts = nc.dram_tensor("consts", [P, 2 * R], F32,
                            kind="ExternalInput").ap()
    out_s = nc.dram_tensor("out_s", [P, 2], F32, kind="ExternalOutput").ap()

    with TileContext(nc) as tc:
        with (
            tc.tile_pool(name="const", bufs=1) as const_pool,
            tc.tile_pool(name="u", bufs=3) as u_pool,
            tc.tile_pool(name="r8", bufs=3) as r8_pool,
            tc.tile_pool(name="sgn", bufs=3) as sgn_pool,
            tc.tile_pool(name="idx", bufs=1) as idx_pool,
            tc.tile_pool(name="epi", bufs=1) as epi_pool,
            tc.tile_pool(name="psum", bufs=2, space="PSUM") as psum_pool,
        ):
            augs_sb = const_pool.tile([KSPLIT, 4 * N], BF16, tag="augs")
            nc.sync.dma_start(augs_sb[:], augs)

            def aug(i):
                return augs_sb[:, i * N:(i + 1) * N]

            consts_sb = const_pool.tile([P, 2 * R], F32, tag="consts")
            nc.sync.dma_start(consts_sb[:], consts)

            dummy = const_pool.tile([P, 1], F32, tag="dummy")
            nc.vector.memset(dummy[:], 0.0)

            idxg_f = {1: idx_pool.tile([P, NCH], F32, tag="ig1", name="ig1"),
                      2: idx_pool.tile([P, NCH], F32, tag="ig2", name="ig2")}
            idxg_i = {1: idx_pool.tile([P, NCH], mybir.dt.int32, tag="ii1",
                                       name="ii1"),
                      2: idx_pool.tile([P, NCH], mybir.dt.int32, tag="ii2",
                                       name="ii2")}
            # gathered candidate groups, flat [P, NCH * R * 3]
            cand = {1: epi_pool.tile([P, NCH * R * 3], F32, tag="cand1",
                                     name="cand1"),
                    2: epi_pool.tile([P, NCH * R * 3], F32, tag="cand2",
                                     name="cand2")}

            for dirn in (1, 2):
                lhsT_all = aug(0) if dirn == 1 else aug(2)
                rhs_all = aug(1) if dirn == 1 else aug(3)
                gsrc = yr if dirn == 1 else xr
                gsrc_g = gsrc.rearrange("(g k) d -> g (k d)", k=R)
                for c in range(NCH):
                    r8 = r8_pool.tile([P, NG], F32, tag="r8")
                    u = u_pool.tile([P, NG], F32, tag="u")
                    for h in range(NH):
                        ps = psum_pool.tile([P, HALF], F32, tag="ps",
                                            space="PSUM")
                        for k in range(HALF // MMFD):
                            m0 = h * HALF + k * MMFD
                            nc.tensor.matmul(
                                ps[:, k * MMFD:(k + 1) * MMFD],
                                lhsT=lhsT_all[:, c * P:(c + 1) * P],
                                rhs=rhs_all[:, m0:m0 + MMFD],
                                start=True, stop=True,
                            )
                        # group maxima (R-wide) of this tile, 1 elem/cyc
                        nc.vector.tensor_reduce(
                            out=u[:, h * GH:(h + 1) * GH],
                            in_=ps[:].rearrange("p (g k) -> p g k", k=R),
                            axis=mybir.AxisListType.X,
                            op=ALU.max,
                        )
                    # prefix max over groups (2 cyc/elem, N/R elems)
                    nc.vector.tensor_tensor_scan(
                        out=r8[:],
                        data0=u[:],
                        data1=dummy[:, 0:1].to_broadcast([P, NG]),
                        initial=NEG_BIG,
                        op0=ALU.max,
                        op1=ALU.bypass,
                    )
                    # group index of first attainment of the row max
                    sgn = sgn_pool.tile([P, NG], BF16, tag="sgn")
                    nc.scalar.activation(
                        out=sgn[:], in_=r8[:, :],
                        func=AF.Sign,
                        bias=r8[:, NG - 1:NG],
                        scale=-1.0,
                        accum_out=idxg_f[dirn][:, c:c + 1],
                    )
                    nc.scalar.activation(
                        out=idxg_i[dirn][:, c:c + 1],
                        in_=idxg_f[dirn][:, c:c + 1],
                        func=AF.Copy, bias=0.0, scale=1.0)
                    # gather the 16-candidate group (48 contiguous floats)
                    nc.gpsimd.indirect_dma_start(
                        out=cand[dirn][:, c * R * 3:(c + 1) * R * 3],
                        out_offset=None,
                        in_=gsrc_g,
                        in_offset=bass.IndirectOffsetOnAxis(
                            ap=idxg_i[dirn][:, c:c + 1], axis=0),
                    )

            # ---- epilogue: exact within-group argmin + sum |diff|^5 ----
            partials = epi_pool.tile([P, 2], F32, tag="partials")
            FC = NCH * R * 3   # candidate floats per partition
            FK = NCH * R       # candidates per partition
            for dirn in (1, 2):
                own = xr if dirn == 1 else yr
                cd = cand[dirn]

                ow = epi_pool.tile([P, NCH, 3], F32, tag=f"ow{dirn}",
                                   name=f"ow{dirn}")
                nc.sync.dma_start(
                    ow[:], own.rearrange("(c p) d -> p c d", p=P))
                # own broadcast over the R candidates: [P, NCH, R, 3]
                owb = bass.AP(ow[:].tensor, ow[:].offset,
                              [ow[:].ap[0], [3, NCH], [0, R], [1, 3]])

                diff = epi_pool.tile([P, FC], F32, tag=f"df{dirn}",
                                     name=f"df{dirn}")
                nc.vector.tensor_sub(
                    diff[:].rearrange("p (c k d) -> p c k d", k=R, d=3),
                    owb, cd[:].rearrange("p (c k d) -> p c k d", k=R, d=3))
                ad = epi_pool.tile([P, FC], F32, tag=f"ab{dirn}",
                                   name=f"ab{dirn}")
                nc.scalar.activation(out=ad[:], in_=diff[:], func=AF.Abs,
                                     bias=0.0, scale=1.0)
                sq = epi_pool.tile([P, FC], F32, tag=f"sq{dirn}",
                                   name=f"sq{dirn}")
                nc.scalar.activation(out=sq[:], in_=ad[:], func=AF.Square,
                                     bias=0.0, scale=1.0)
                # squared L2 distance per candidate [P, NCH*R]
                dd = epi_pool.tile([P, FK], F32, tag=f"dd{dirn}",
                                   name=f"dd{dirn}")
                nc.vector.tensor_reduce(
                    out=dd[:], in_=sq[:].rearrange("p (k d) -> p k d", d=3),
                    axis=mybir.AxisListType.X, op=ALU.add)
                # min distance per row [P, NCH]
                dmin = epi_pool.tile([P, NCH], F32, tag=f"dm{dirn}",
                                     name=f"dm{dirn}")
                nc.vector.tensor_reduce(
                    out=dmin[:], in_=dd[:].rearrange("p (c k) -> p c k", k=R),
                    axis=mybir.AxisListType.X, op=ALU.min)
                dminb = bass.AP(dmin[:].tensor, dmin[:].offset,
                                [dmin[:].ap[0], [1, NCH], [0, R]])
                mask = epi_pool.tile([P, FK], F32, tag=f"mk{dirn}",
                                     name=f"mk{dirn}")
                nc.vector.tensor_tensor(
                    out=mask[:].rearrange("p (c k) -> p c k", k=R),
                    in0=dd[:].rearrange("p (c k) -> p c k", k=R),
                    in1=dminb, op=ALU.is_le)
                # first-attaining candidate: k* = min_k (iota_k + 16*(1-mask))
                iotap = bass.AP(consts_sb[:].tensor, consts_sb[:].offset + R,
                               [consts_sb[:].ap[0], [0, NCH], [1, R]])
                tk = epi_pool.tile([P, FK], F32, tag=f"tk{dirn}",
                                   name=f"tk{dirn}")
                nc.vector.scalar_tensor_tensor(
                    out=tk[:].rearrange("p (c k) -> p c k", k=R),
                    in0=mask[:].rearrange("p (c k) -> p c k", k=R),
                    scalar=-float(R), op0=ALU.mult,
                    in1=iotap, op1=ALU.add)
                kstar = epi_pool.tile([P, NCH], F32, tag=f"ks{dirn}",
                                      name=f"ks{dirn}")
                nc.vector.tensor_reduce(
                    out=kstar[:], in_=tk[:].rearrange("p (c k) -> p c k", k=R),
                    axis=mybir.AxisListType.X, op=ALU.min)
                # kstar is iota[k*]+R-R... note tk = iota+R-16*mask so the
                # masked entries are iota exactly; min = k*.
                ksb = bass.AP(kstar[:].tensor, kstar[:].offset,
                              [kstar[:].ap[0], [1, NCH], [0, R]])
                onehot = epi_pool.tile([P, FK], F32, tag=f"oh{dirn}",
                                       name=f"oh{dirn}")
                nc.vector.tensor_tensor(
                    out=onehot[:].rearrange("p (c k) -> p c k", k=R),
                    in0=bass.AP(consts_sb[:].tensor, consts_sb[:].offset,
                                [consts_sb[:].ap[0], [0, NCH], [1, R]]),
                    in1=ksb, op=ALU.is_equal)
                # p5 per candidate, then select the winner
                q4 = epi_pool.tile([P, FC], F32, tag=f"q4{dirn}",
                                   name=f"q4{dirn}")
                nc.scalar.activation(out=q4[:], in_=sq[:], func=AF.Square,
                                     bias=0.0, scale=1.0)
                p5e = epi_pool.tile([P, FC], F32, tag=f"p5{dirn}",
                                    name=f"p5{dirn}")
                nc.gpsimd.tensor_mul(p5e[:], q4[:], ad[:])
                p5k = epi_pool.tile([P, FK], F32, tag=f"pk{dirn}",
                                    name=f"pk{dirn}")
                nc.vector.tensor_reduce(
                    out=p5k[:], in_=p5e[:].rearrange("p (k d) -> p k d", d=3),
                    axis=mybir.AxisListType.X, op=ALU.add)
                psel = epi_pool.tile([P, FK], F32, tag=f"pl{dirn}",
                                     name=f"pl{dirn}")
                nc.gpsimd.tensor_mul(psel[:], p5k[:], onehot[:])
                nc.vector.reduce_sum(partials[:, dirn - 1:dirn], psel[:],
                                     axis=mybir.AxisListType.X)

            nc.sync.dma_start(out_s, partials[:])

    nc.compile()
    return nc


def _to_bf16(a):
    return a.astype(ml_dtypes.bfloat16)


def _split3(a):
    a = np.asarray(a, np.float32)
    h = _to_bf16(a)
    m = _to_bf16(a - h.astype(np.float32))
    l = _to_bf16(a - h.astype(np.float32) - m.astype(np.float32))
    return h, m, l


def _host_prep(xb, yb):
    xb = np.ascontiguousarray(xb, dtype=np.float32)
    yb = np.ascontiguousarray(yb, dtype=np.float32)
    n = xb.shape[0]
    ones = np.ones((n,), np.float32)

    def build(sta, mov, key_sq):
        """bf16 split terms for s = sum_c sta_c * (2 mov_c) - |mov|^2
        as seen with `sta` stationary; key_sq = -(|mov|^2)."""
        ta, tb = [], []
        for c in range(3):
            a, b = _split3(sta[:, c]), _split3(2.0 * mov[:, c])
            for i, j in ((0, 0), (0, 1), (0, 2), (1, 0), (1, 1), (2, 0)):
                ta.append(a[i])
                tb.append(b[j])
        sh, sm, sl = _split3(key_sq)
        ob = _to_bf16(ones)
        for s in (sh, sm, sl):
            ta.append(ob)
            tb.append(s)
        A = np.stack(ta).astype(ml_dtypes.bfloat16)
        Bm = np.stack(tb).astype(ml_dtypes.bfloat16)
        return A, Bm

    y2 = -(yb * yb).sum(-1)
    x2 = -(xb * xb).sum(-1)
    A1, B1 = build(xb, yb, y2)   # dir 1: lhsT = x terms, rhs = y terms
    A2, B2 = build(yb, xb, x2)   # dir 2: lhsT = y terms, rhs = x terms

    augs = np.empty((KSPLIT, 4 * n), ml_dtypes.bfloat16)
    augs[:, 0 * n:1 * n] = A1
    augs[:, 1 * n:2 * n] = B1
    augs[:, 2 * n:3 * n] = A2
    augs[:, 3 * n:4 * n] = B2

    iota = np.arange(R, dtype=np.float32)
    consts = np.tile(np.concatenate([iota, iota + R])[None, :], (P, 1))
    return {"augs": augs, "xr": xb, "yr": yb,
            "consts": np.ascontiguousarray(consts, np.float32)}


_NC = None


def _get_nc():
    global _NC
    if _NC is None:
        _NC = _build_nc()
    return _NC


def run_on_hw(x, y, **spmd_kwargs):
    """Run the SPMD kernel; returns (per-core out arrays, BassKernelResults)."""
    x = np.asarray(x, dtype=np.float32)
    y = np.asarray(y, dtype=np.float32)
    assert x.shape == (B, N_FULL, 3) and y.shape == (B, N_FULL, 3)
    nc = _get_nc()
    in_maps = [_host_prep(x[b], y[b]) for b in range(B)]
    res = bass_utils.run_bass_kernel_spmd(
        nc, in_maps, core_ids=list(range(B)), **spmd_kwargs)
    outs = [res.results[b]["out_s"] for b in range(B)]
    return outs, res


def kernel(x, y):
    outs, _ = run_on_hw(x, y)
    vals = []
    for o in outs:
        s = np.asarray(o, dtype=np.float64).sum(axis=0)
        vals.append(s[0] ** 0.2 + s[1] ** 0.2)
    return np.float32(np.mean(vals))


# revision 16
# speedup vs baseline: 2.4742x; 1.0108x over previous
"""Chamfer p=5 loss (nn_ChamferLossP) — Bass kernel for 8x TRN2 NeuronCores.

Sharding: data-parallel over the batch dim B=8, one batch per core; host
combines the per-core partial sums (the final "mean all-reduce").

Per-core device algorithm (direction 1 shown; direction 2 swaps x<->y):

  argmin_m ||x_n - y_m||^2  ==  argmax_m s[n,m],  s = 2 x.y - |y_m|^2.

  The PE materialises s in PSUM tiles [128n x 512m] with a single bf16
  matmul per tile: each fp32 factor is split into 3 bf16 limbs and the
  6 significant limb products per coordinate (plus 3 limbs of the -|y|^2
  term) form a 21-term contraction — fp32-accurate keys (~1e-7 rel) at
  bf16 speed (4x faster than the PE's multi-pass fp32 mode).

  Index extraction is two-level: the DVE reduces each PSUM tile to
  16-element group maxima (1 elem/cycle), then a short tensor_tensor_scan
  (running max, 2 cycles/elem but only N/16 elems) produces the prefix
  maxima r8 whose last column is the row max g.  The Scalar engine's
  Sign(g - r8) with accum_out counts groups strictly before the first
  attainment of g — the exact first-attainment group index (Sign(0)=0 on
  HW, probed).  One indirect DMA per 128-row chunk gathers that group's
  16 candidate points (48 contiguous floats); the epilogue recomputes the
  16 exact fp32 squared distances, picks the winner (first index on
  ties, matching jnp.argmin), and accumulates sum_c |x - nn|^5.
"""

import numpy as np
import ml_dtypes

import concourse.bass as bass
import concourse.bacc as bacc
import concourse.mybir as mybir
from concourse import bass_utils
from concourse.tile import TileContext

F32 = mybir.dt.float32
BF16 = mybir.dt.bfloat16
AF = mybir.ActivationFunctionType
ALU = mybir.AluOpType

B = 8
N_FULL = 4096
HALF_FULL = 2048
P = 128
R = 16              # argmin group size (candidates per gather)
KSPLIT = 21         # bf16 split-contraction terms
NEG_BIG = -3.0e38


def _build_nc(N=N_FULL, HALF=HALF_FULL, num_devices=B):
    NCH = N // P         # 128-row chunks per direction
    MMFD = min(512, HALF)
    NH = N // HALF       # psum tiles per chunk
    NG = N // R          # groups per row
    GH = HALF // R       # groups per psum tile

    nc = bacc.Bacc("TRN2", target_bir_lowering=False,
                   num_devices=num_devices)

    # augs columns: [x1_lhsT | y1_rhs | y2_lhsT | x2_rhs], each N wide, bf16.
    augs = nc.dram_tensor("augs", [KSPLIT, 4 * N], BF16,
                          kind="ExternalInput").ap()
    xr = nc.dram_tensor("xr", [N, 3], F32, kind="ExternalInput").ap()
    yr = nc.dram_tensor("yr", [N, 3], F32, kind="ExternalInput").ap()
    # consts row: [iota16 | iota16 + R]
    consts = nc.dram_tensor("consts", [P, 2 * R], F32,
                            kind="ExternalInput").ap()
    out_s = nc.dram_tensor("out_s", [P, 2], F32, kind="ExternalOutput").ap()

    with TileContext(nc) as tc:
        with (
            tc.tile_pool(name="const", bufs=1) as const_pool,
            tc.tile_pool(name="u", bufs=3) as u_pool,
            tc.tile_pool(name="r8", bufs=3) as r8_pool,
            tc.tile_pool(name="sgn", bufs=3) as sgn_pool,
            tc.tile_pool(name="idx", bufs=1) as idx_pool,
            tc.tile_pool(name="epi", bufs=1) as epi_pool,
            tc.tile_pool(name="psum", bufs=2, space="PSUM") as psum_pool,
        ):
            augs_sb = const_pool.tile([KSPLIT, 4 * N], BF16, tag="augs")
            # split the load so direction 1's operands arrive first
            for i in range(4):
                nc.sync.dma_start(augs_sb[:, i * N:(i + 1) * N],
                                  augs[:, i * N:(i + 1) * N])

            def aug(i):
                return augs_sb[:, i * N:(i + 1) * N]

            consts_sb = const_pool.tile([P, 2 * R], F32, tag="consts")
            nc.sync.dma_start(consts_sb[:], consts)

            dummy = const_pool.tile([P, 1], F32, tag="dummy")
            nc.vector.memset(dummy[:], 0.0)

            idxg_f = {1: idx_pool.tile([P, NCH], F32, tag="ig1", name="ig1"),
                      2: idx_pool.tile([P, NCH], F32, tag="ig2", name="ig2")}
            idxg_i = {1: idx_pool.tile([P, NCH], mybir.dt.int32, tag="ii1",
                                       name="ii1"),
                      2: idx_pool.tile([P, NCH], mybir.dt.int32, tag="ii2",
                                       name="ii2")}
            # gathered candidate groups, flat [P, NCH * R * 3]
            cand = {1: epi_pool.tile([P, NCH * R * 3], F32, tag="cand1",
                                     name="cand1"),
                    2: epi_pool.tile([P, NCH * R * 3], F32, tag="cand2",
                                     name="cand2")}

            for dirn in (1, 2):
                lhsT_all = aug(0) if dirn == 1 else aug(2)
                rhs_all = aug(1) if dirn == 1 else aug(3)
                gsrc = yr if dirn == 1 else xr
                gsrc_g = gsrc.rearrange("(g k) d -> g (k d)", k=R)
                for c in range(NCH):
                    r8 = r8_pool.tile([P, NG], F32, tag="r8")
                    u = u_pool.tile([P, NG], F32, tag="u")
                    for h in range(NH):
                        ps = psum_pool.tile([P, HALF], F32, tag="ps",
                                            space="PSUM")
                        for k in range(HALF // MMFD):
                            m0 = h * HALF + k * MMFD
                            nc.tensor.matmul(
                                ps[:, k * MMFD:(k + 1) * MMFD],
                                lhsT=lhsT_all[:, c * P:(c + 1) * P],
                                rhs=rhs_all[:, m0:m0 + MMFD],
                                start=True, stop=True,
                            )
                        # group maxima (R-wide) of this tile, 1 elem/cyc
                        nc.vector.tensor_reduce(
                            out=u[:, h * GH:(h + 1) * GH],
                            in_=ps[:].rearrange("p (g k) -> p g k", k=R),
                            axis=mybir.AxisListType.X,
                            op=ALU.max,
                        )
                    # prefix max over groups (2 cyc/elem, N/R elems)
                    nc.vector.tensor_tensor_scan(
                        out=r8[:],
                        data0=u[:],
                        data1=dummy[:, 0:1].to_broadcast([P, NG]),
                        initial=NEG_BIG,
                        op0=ALU.max,
                        op1=ALU.bypass,
                    )
                    # group index of first attainment of the row max
                    sgn = sgn_pool.tile([P, NG], BF16, tag="sgn")
                    nc.scalar.activation(
                        out=sgn[:], in_=r8[:, :],
                        func=AF.Sign,
                        bias=r8[:, NG - 1:NG],
                        scale=-1.0,
                        accum_out=idxg_f[dirn][:, c:c + 1],
                    )
                    nc.scalar.activation(
                        out=idxg_i[dirn][:, c:c + 1],
                        in_=idxg_f[dirn][:, c:c + 1],
                        func=AF.Copy, bias=0.0, scale=1.0)
                    # gather the 16-candidate group (48 contiguous floats)
                    nc.gpsimd.indirect_dma_start(
                        out=cand[dirn][:, c * R * 3:(c + 1) * R * 3],
                        out_offset=None,
                        in_=gsrc_g,
                        in_offset=bass.IndirectOffsetOnAxis(
                            ap=idxg_i[dirn][:, c:c + 1], axis=0),
                    )

            # ---- epilogue: exact within-group argmin + sum |diff|^5 ----
            partials = epi_pool.tile([P, 2], F32, tag="partials")
            FC = NCH * R * 3   # candidate floats per partition
            FK = NCH * R       # candidates per partition
            for dirn in (1, 2):
                own = xr if dirn == 1 else yr
                cd = cand[dirn]

                ow = epi_pool.tile([P, NCH, 3], F32, tag=f"ow{dirn}",
                                   name=f"ow{dirn}")
                nc.sync.dma_start(
                    ow[:], own.rearrange("(c p) d -> p c d", p=P))
                # own broadcast over the R candidates: [P, NCH, R, 3]
                owb = bass.AP(ow[:].tensor, ow[:].offset,
                              [ow[:].ap[0], [3, NCH], [0, R], [1, 3]])

                diff = epi_pool.tile([P, FC], F32, tag=f"df{dirn}",
                                     name=f"df{dirn}")
                nc.vector.tensor_sub(
                    diff[:].rearrange("p (c k d) -> p c k d", k=R, d=3),
                    owb, cd[:].rearrange("p (c k d) -> p c k d", k=R, d=3))
                ad = epi_pool.tile([P, FC], F32, tag=f"ab{dirn}",
                                   name=f"ab{dirn}")
                nc.scalar.activation(out=ad[:], in_=diff[:], func=AF.Abs,
                                     bias=0.0, scale=1.0)
                sq = epi_pool.tile([P, FC], F32, tag=f"sq{dirn}",
                                   name=f"sq{dirn}")
                nc.scalar.activation(out=sq[:], in_=ad[:], func=AF.Square,
                                     bias=0.0, scale=1.0)
                # squared L2 distance per candidate [P, NCH*R]
                dd = epi_pool.tile([P, FK], F32, tag=f"dd{dirn}",
                                   name=f"dd{dirn}")
                nc.vector.tensor_reduce(
                    out=dd[:], in_=sq[:].rearrange("p (k d) -> p k d", d=3),
                    axis=mybir.AxisListType.X, op=ALU.add)
                # min distance per row [P, NCH]
                dmin = epi_pool.tile([P, NCH], F32, tag=f"dm{dirn}",
                                     name=f"dm{dirn}")
                nc.vector.tensor_reduce(
                    out=dmin[:], in_=dd[:].rearrange("p (c k) -> p c k", k=R),
                    axis=mybir.AxisListType.X, op=ALU.min)
                dminb = bass.AP(dmin[:].tensor, dmin[:].offset,
                                [dmin[:].ap[0], [1, NCH], [0, R]])
                mask = epi_pool.tile([P, FK], F32, tag=f"mk{dirn}",
                                     name=f"mk{dirn}")
                nc.vector.tensor_tensor(
                    out=mask[:].rearrange("p (c k) -> p c k", k=R),
                    in0=dd[:].rearrange("p (c k) -> p c k", k=R),
                    in1=dminb, op=ALU.is_le)
                # first-attaining candidate: k* = min_k (iota_k + 16*(1-mask))
                iotap = bass.AP(consts_sb[:].tensor, consts_sb[:].offset + R,
                               [consts_sb[:].ap[0], [0, NCH], [1, R]])
                tk = epi_pool.tile([P, FK], F32, tag=f"tk{dirn}",
                                   name=f"tk{dirn}")
                nc.vector.scalar_tensor_tensor(
                    out=tk[:].rearrange("p (c k) -> p c k", k=R),
                    in0=mask[:].rearrange("p (c k) -> p c k", k=R),
                    scalar=-float(R), op0=ALU.mult,
                    in1=iotap, op1=ALU.add)
                kstar = epi_pool.tile([P, NCH], F32, tag=f"ks{dirn}",
                                      name=f"ks{dirn}")
                nc.vector.tensor_reduce(
                    out=kstar[:], in_=tk[:].rearrange("p (c k) -> p c k", k=R),
                    axis=mybir.AxisListType.X, op=ALU.min)
                # kstar is iota[k*]+R-R... note tk = iota+R-16*mask so the
                # masked entries are iota exactly; min = k*.
                ksb = bass.AP(kstar[:].tensor, kstar[:].offset,
                              [kstar[:].ap[0], [1, NCH], [0, R]])
                onehot = epi_pool.tile([P, FK], F32, tag=f"oh{dirn}",
                                       name=f"oh{dirn}")
                nc.vector.tensor_tensor(
                    out=onehot[:].rearrange("p (c k) -> p c k", k=R),
                    in0=bass.AP(consts_sb[:].tensor, consts_sb[:].offset,
                                [consts_sb[:].ap[0], [0, NCH], [1, R]]),
                    in1=ksb, op=ALU.is_equal)
                # p5 per candidate, then select the winner
                q4 = epi_pool.tile([P, FC], F32, tag=f"q4{dirn}",
                                   name=f"q4{dirn}")
                nc.scalar.activation(out=q4[:], in_=sq[:], func=AF.Square,
                                     bias=0.0, scale=1.0)
                p5e = epi_pool.tile([P, FC], F32, tag=f"p5{dirn}",
                                    name=f"p5{dirn}")
                nc.gpsimd.tensor_mul(p5e[:], q4[:], ad[:])
                p5k = epi_pool.tile([P, FK], F32, tag=f"pk{dirn}",
                                    name=f"pk{dirn}")
                nc.vector.tensor_reduce(
                    out=p5k[:], in_=p5e[:].rearrange("p (k d) -> p k d", d=3),
                    axis=mybir.AxisListType.X, op=ALU.add)
                psel = epi_pool.tile([P, FK], F32, tag=f"pl{dirn}",
                                     name=f"pl{dirn}")
                nc.gpsimd.tensor_mul(psel[:], p5k[:], onehot[:])
                nc.vector.reduce_sum(partials[:, dirn - 1:dirn], psel[:],
                                     axis=mybir.AxisListType.X)

            nc.sync.dma_start(out_s, partials[:])

    nc.compile()
    return nc


def _to_bf16(a):
    return a.astype(ml_dtypes.bfloat16)


def _split3(a):
    a = np.asarray(a, np.float32)
    h = _to_bf16(a)
    m = _to_bf16(a - h.astype(np.float32))
    l = _to_bf16(a - h.astype(np.float32) - m.astype(np.float32))
    return h, m, l


def _host_prep(xb, yb):
    xb = np.ascontiguousarray(xb, dtype=np.float32)
    yb = np.ascontiguousarray(yb, dtype=np.float32)
    n = xb.shape[0]
    ones = np.ones((n,), np.float32)

    def build(sta, mov, key_sq):
        """bf16 split terms for s = sum_c sta_c * (2 mov_c) - |mov|^2
        as seen with `sta` stationary; key_sq = -(|mov|^2)."""
        ta, tb = [], []
        for c in range(3):
            a, b = _split3(sta[:, c]), _split3(2.0 * mov[:, c])
            for i, j in ((0, 0), (0, 1), (0, 2), (1, 0), (1, 1), (2, 0)):
                ta.append(a[i])
                tb.append(b[j])
        sh, sm, sl = _split3(key_sq)
        ob = _to_bf16(ones)
        for s in (sh, sm, sl):
            ta.append(ob)
            tb.append(s)
        A = np.stack(ta).astype(ml_dtypes.bfloat16)
        Bm = np.stack(tb).astype(ml_dtypes.bfloat16)
        return A, Bm

    y2 = -(yb * yb).sum(-1)
    x2 = -(xb * xb).sum(-1)
    A1, B1 = build(xb, yb, y2)   # dir 1: lhsT = x terms, rhs = y terms
    A2, B2 = build(yb, xb, x2)   # dir 2: lhsT = y terms, rhs = x terms

    augs = np.empty((KSPLIT, 4 * n), ml_dtypes.bfloat16)
    augs[:, 0 * n:1 * n] = A1
    augs[:, 1 * n:2 * n] = B1
    augs[:, 2 * n:3 * n] = A2
    augs[:, 3 * n:4 * n] = B2

    iota = np.arange(R, dtype=np.float32)
    consts = np.tile(np.concatenate([iota, iota + R])[None, :], (P, 1))
    return {"augs": augs, "xr": xb, "yr": yb,
            "consts": np.ascontiguousarray(consts, np.float32)}


_NC = None


def _get_nc():
    global _NC
    if _NC is None:
        _NC = _build_nc()
    return _NC


def run_on_hw(x, y, **spmd_kwargs):
    """Run the SPMD kernel; returns (per-core out arrays, BassKernelResults)."""
    x = np.asarray(x, dtype=np.float32)
    y = np.asarray(y, dtype=np.float32)
    assert x.shape == (B, N_FULL, 3) and y.shape == (B, N_FULL, 3)
    nc = _get_nc()
    in_maps = [_host_prep(x[b], y[b]) for b in range(B)]
    res = bass_utils.run_bass_kernel_spmd(
        nc, in_maps, core_ids=list(range(B)), **spmd_kwargs)
    outs = [res.results[b]["out_s"] for b in range(B)]
    return outs, res


def kernel(x, y):
    outs, _ = run_on_hw(x, y)
    vals = []
    for o in outs:
        s = np.asarray(o, dtype=np.float64).sum(axis=0)
        vals.append(s[0] ** 0.2 + s[1] ** 0.2)
    return np.float32(np.mean(vals))


# revision 20
# speedup vs baseline: 2.4772x; 1.0012x over previous
"""Chamfer p=5 loss (nn_ChamferLossP) — Bass kernel for 8x TRN2 NeuronCores.

Sharding: data-parallel over the batch dim B=8, one batch per core; host
combines the per-core partial sums (the final "mean all-reduce").

Per-core device algorithm (direction 1 shown; direction 2 swaps x<->y):

  argmin_m ||x_n - y_m||^2  ==  argmax_m s[n,m],  s = 2 x.y - |y_m|^2.

  The PE materialises s in PSUM tiles [128n x 512m] with a single bf16
  matmul per tile: each fp32 factor is split into 3 bf16 limbs and the
  6 significant limb products per coordinate (plus 3 limbs of the -|y|^2
  term) form a 21-term contraction — fp32-accurate keys (~1e-7 rel) at
  bf16 speed (4x faster than the PE's multi-pass fp32 mode).

  Index extraction is two-level: the DVE reduces each PSUM tile to
  16-element group maxima (1 elem/cycle), then a short tensor_tensor_scan
  (running max, 2 cycles/elem but only N/16 elems) produces the prefix
  maxima r8 whose last column is the row max g.  The Scalar engine's
  Sign(g - r8) with accum_out counts groups strictly before the first
  attainment of g — the exact first-attainment group index (Sign(0)=0 on
  HW, probed).  One indirect DMA per 128-row chunk gathers that group's
  16 candidate points (48 contiguous floats); the epilogue recomputes the
  16 exact fp32 squared distances, picks the winner (first index on
  ties, matching jnp.argmin), and accumulates sum_c |x - nn|^5.
"""

import numpy as np
import ml_dtypes

import concourse.bass as bass
import concourse.bacc as bacc
import concourse.mybir as mybir
from concourse import bass_utils
from concourse.tile import TileContext

F32 = mybir.dt.float32
BF16 = mybir.dt.bfloat16
AF = mybir.ActivationFunctionType
ALU = mybir.AluOpType

B = 8
N_FULL = 4096
HALF_FULL = 2048
P = 128
R = 16              # argmin group size (candidates per gather)
KSPLIT = 21         # bf16 split-contraction terms
NEG_BIG = -3.0e38


def _build_nc(N=N_FULL, HALF=HALF_FULL, num_devices=B):
    NCH = N // P         # 128-row chunks per direction
    MMFD = min(512, HALF)
    NH = N // HALF       # psum tiles per chunk
    NG = N // R          # groups per row
    GH = HALF // R       # groups per psum tile

    nc = bacc.Bacc("TRN2", target_bir_lowering=False,
                   num_devices=num_devices)

    # augs columns: [x1_lhsT | y1_rhs | y2_lhsT | x2_rhs], each N wide, bf16.
    augs = nc.dram_tensor("augs", [KSPLIT, 4 * N], BF16,
                          kind="ExternalInput").ap()
    xr = nc.dram_tensor("xr", [N, 3], F32, kind="ExternalInput").ap()
    yr = nc.dram_tensor("yr", [N, 3], F32, kind="ExternalInput").ap()
    # consts row: [iota16 | iota16 + R]
    consts = nc.dram_tensor("consts", [P, 2 * R], F32,
                            kind="ExternalInput").ap()
    out_s = nc.dram_tensor("out_s", [P, 2], F32, kind="ExternalOutput").ap()

    with TileContext(nc) as tc:
        with (
            tc.tile_pool(name="const", bufs=1) as const_pool,
            tc.tile_pool(name="u", bufs=3) as u_pool,
            tc.tile_pool(name="r8", bufs=3) as r8_pool,
            tc.tile_pool(name="sgn", bufs=3) as sgn_pool,
            tc.tile_pool(name="idx", bufs=1) as idx_pool,
            tc.tile_pool(name="epi", bufs=1) as epi_pool,
            tc.tile_pool(name="psum", bufs=2, space="PSUM") as psum_pool,
        ):
            augs_sb = const_pool.tile([KSPLIT, 4 * N], BF16, tag="augs")
            # split the load so direction 1's operands arrive first
            for i in range(4):
                nc.sync.dma_start(augs_sb[:, i * N:(i + 1) * N],
                                  augs[:, i * N:(i + 1) * N])

            def aug(i):
                return augs_sb[:, i * N:(i + 1) * N]

            consts_sb = const_pool.tile([P, 2 * R], F32, tag="consts")
            nc.sync.dma_start(consts_sb[:], consts)

            dummy = const_pool.tile([P, 1], F32, tag="dummy")
            nc.vector.memset(dummy[:], 0.0)

            idxg_f = {1: idx_pool.tile([P, NCH], F32, tag="ig1", name="ig1"),
                      2: idx_pool.tile([P, NCH], F32, tag="ig2", name="ig2")}
            idxg_i = {1: idx_pool.tile([P, NCH], mybir.dt.int32, tag="ii1",
                                       name="ii1"),
                      2: idx_pool.tile([P, NCH], mybir.dt.int32, tag="ii2",
                                       name="ii2")}
            # gathered candidate groups, flat [P, NCH * R * 3]
            cand = {1: epi_pool.tile([P, NCH * R * 3], F32, tag="cand1",
                                     name="cand1"),
                    2: epi_pool.tile([P, NCH * R * 3], F32, tag="cand2",
                                     name="cand2")}

            for dirn in (1, 2):
                lhsT_all = aug(0) if dirn == 1 else aug(2)
                rhs_all = aug(1) if dirn == 1 else aug(3)
                gsrc = yr if dirn == 1 else xr
                gsrc_g = gsrc.rearrange("(g k) d -> g (k d)", k=R)
                for c in range(NCH):
                    r8 = r8_pool.tile([P, NG], F32, tag="r8")
                    u = u_pool.tile([P, NG], F32, tag="u")
                    for h in range(NH):
                        ps = psum_pool.tile([P, HALF], F32, tag="ps",
                                            space="PSUM")
                        for k in range(HALF // MMFD):
                            m0 = h * HALF + k * MMFD
                            nc.tensor.matmul(
                                ps[:, k * MMFD:(k + 1) * MMFD],
                                lhsT=lhsT_all[:, c * P:(c + 1) * P],
                                rhs=rhs_all[:, m0:m0 + MMFD],
                                start=True, stop=True,
                            )
                        # group maxima (R-wide) of this tile, 1 elem/cyc
                        nc.vector.tensor_reduce(
                            out=u[:, h * GH:(h + 1) * GH],
                            in_=ps[:].rearrange("p (g k) -> p g k", k=R),
                            axis=mybir.AxisListType.X,
                            op=ALU.max,
                        )
                    # prefix max over groups (2 cyc/elem, N/R elems)
                    nc.vector.tensor_tensor_scan(
                        out=r8[:],
                        data0=u[:],
                        data1=dummy[:, 0:1].to_broadcast([P, NG]),
                        initial=NEG_BIG,
                        op0=ALU.max,
                        op1=ALU.bypass,
                    )
                    # group index of first attainment of the row max
                    sgn = sgn_pool.tile([P, NG], BF16, tag="sgn")
                    nc.scalar.activation(
                        out=sgn[:], in_=r8[:, :],
                        func=AF.Sign,
                        bias=r8[:, NG - 1:NG],
                        scale=-1.0,
                        accum_out=idxg_f[dirn][:, c:c + 1],
                    )
                    nc.scalar.activation(
                        out=idxg_i[dirn][:, c:c + 1],
                        in_=idxg_f[dirn][:, c:c + 1],
                        func=AF.Copy, bias=0.0, scale=1.0)
                    # gather the 16-candidate group (48 contiguous floats)
                    nc.gpsimd.indirect_dma_start(
                        out=cand[dirn][:, c * R * 3:(c + 1) * R * 3],
                        out_offset=None,
                        in_=gsrc_g,
                        in_offset=bass.IndirectOffsetOnAxis(
                            ap=idxg_i[dirn][:, c:c + 1], axis=0),
                    )

            # ---- epilogue: exact within-group argmin + sum |diff|^5 ----
            partials = epi_pool.tile([P, 2], F32, tag="partials")
            FC = NCH * R * 3   # candidate floats per partition
            FK = NCH * R       # candidates per partition
            for dirn in (1, 2):
                own = xr if dirn == 1 else yr
                cd = cand[dirn]

                ow = epi_pool.tile([P, NCH, 3], F32, tag=f"ow{dirn}",
                                   name=f"ow{dirn}")
                nc.sync.dma_start(
                    ow[:], own.rearrange("(c p) d -> p c d", p=P))
                # own broadcast over the R candidates: [P, NCH, R, 3]
                owb = bass.AP(ow[:].tensor, ow[:].offset,
                              [ow[:].ap[0], [3, NCH], [0, R], [1, 3]])

                diff = epi_pool.tile([P, FC], F32, tag=f"df{dirn}",
                                     name=f"df{dirn}")
                nc.vector.tensor_sub(
                    diff[:].rearrange("p (c k d) -> p c k d", k=R, d=3),
                    owb, cd[:].rearrange("p (c k d) -> p c k d", k=R, d=3))
                ad = epi_pool.tile([P, FC], F32, tag=f"ab{dirn}",
                                   name=f"ab{dirn}")
                nc.scalar.activation(out=ad[:], in_=diff[:], func=AF.Abs,
                                     bias=0.0, scale=1.0)
                sq = epi_pool.tile([P, FC], F32, tag=f"sq{dirn}",
                                   name=f"sq{dirn}")
                nc.scalar.activation(out=sq[:], in_=ad[:], func=AF.Square,
                                     bias=0.0, scale=1.0)
                # squared L2 distance per candidate [P, NCH*R]
                dd = epi_pool.tile([P, FK], F32, tag=f"dd{dirn}",
                                   name=f"dd{dirn}")
                nc.vector.tensor_reduce(
                    out=dd[:], in_=sq[:].rearrange("p (k d) -> p k d", d=3),
                    axis=mybir.AxisListType.X, op=ALU.add)
                # min distance per row [P, NCH]
                dmin = epi_pool.tile([P, NCH], F32, tag=f"dm{dirn}",
                                     name=f"dm{dirn}")
                nc.vector.tensor_reduce(
                    out=dmin[:], in_=dd[:].rearrange("p (c k) -> p c k", k=R),
                    axis=mybir.AxisListType.X, op=ALU.min)
                dminb = bass.AP(dmin[:].tensor, dmin[:].offset,
                                [dmin[:].ap[0], [1, NCH], [0, R]])
                mask = epi_pool.tile([P, FK], F32, tag=f"mk{dirn}",
                                     name=f"mk{dirn}")
                nc.vector.tensor_tensor(
                    out=mask[:].rearrange("p (c k) -> p c k", k=R),
                    in0=dd[:].rearrange("p (c k) -> p c k", k=R),
                    in1=dminb, op=ALU.is_le)
                # first-attaining candidate: k* = min_k (iota_k + 16*(1-mask))
                iotap = bass.AP(consts_sb[:].tensor, consts_sb[:].offset + R,
                               [consts_sb[:].ap[0], [0, NCH], [1, R]])
                tk = epi_pool.tile([P, FK], F32, tag=f"tk{dirn}",
                                   name=f"tk{dirn}")
                nc.vector.scalar_tensor_tensor(
                    out=tk[:].rearrange("p (c k) -> p c k", k=R),
                    in0=mask[:].rearrange("p (c k) -> p c k", k=R),
                    scalar=-float(R), op0=ALU.mult,
                    in1=iotap, op1=ALU.add)
                kstar = epi_pool.tile([P, NCH], F32, tag=f"ks{dirn}",
                                      name=f"ks{dirn}")
                nc.vector.tensor_reduce(
                    out=kstar[:], in_=tk[:].rearrange("p (c k) -> p c k", k=R),
                    axis=mybir.AxisListType.X, op=ALU.min)
                # kstar is iota[k*]+R-R... note tk = iota+R-16*mask so the
                # masked entries are iota exactly; min = k*.
                ksb = bass.AP(kstar[:].tensor, kstar[:].offset,
                              [kstar[:].ap[0], [1, NCH], [0, R]])
                onehot = epi_pool.tile([P, FK], F32, tag=f"oh{dirn}",
                                       name=f"oh{dirn}")
                nc.vector.tensor_tensor(
                    out=onehot[:].rearrange("p (c k) -> p c k", k=R),
                    in0=bass.AP(consts_sb[:].tensor, consts_sb[:].offset,
                                [consts_sb[:].ap[0], [0, NCH], [1, R]]),
                    in1=ksb, op=ALU.is_equal)
                # p5 per candidate, then select the winner
                q4 = epi_pool.tile([P, FC], F32, tag=f"q4{dirn}",
                                   name=f"q4{dirn}")
                nc.scalar.activation(out=q4[:], in_=sq[:], func=AF.Square,
                                     bias=0.0, scale=1.0)
                p5e = epi_pool.tile([P, FC], F32, tag=f"p5{dirn}",
                                    name=f"p5{dirn}")
                nc.gpsimd.tensor_mul(p5e[:], q4[:], ad[:])
                p5k = epi_pool.tile([P, FK], F32, tag=f"pk{dirn}",
                                    name=f"pk{dirn}")
                nc.vector.tensor_reduce(
                    out=p5k[:], in_=p5e[:].rearrange("p (k d) -> p k d", d=3),
                    axis=mybir.AxisListType.X, op=ALU.add)
                psel = epi_pool.tile([P, FK], F32, tag=f"pl{dirn}",
                                     name=f"pl{dirn}")
                nc.gpsimd.tensor_mul(psel[:], p5k[:], onehot[:])
                nc.vector.reduce_sum(partials[:, dirn - 1:dirn], psel[:],
                                     axis=mybir.AxisListType.X)

            nc.sync.dma_start(out_s, partials[:])

    nc.compile()
    return nc


def _to_bf16(a):
    return a.astype(ml_dtypes.bfloat16)


def _split3(a):
    a = np.asarray(a, np.float32)
    h = _to_bf16(a)
    m = _to_bf16(a - h.astype(np.float32))
    l = _to_bf16(a - h.astype(np.float32) - m.astype(np.float32))
    return h, m, l


def _host_prep(xb, yb):
    xb = np.ascontiguousarray(xb, dtype=np.float32)
    yb = np.ascontiguousarray(yb, dtype=np.float32)
    n = xb.shape[0]
    ones = np.ones((n,), np.float32)

    def build(sta, mov, key_sq):
        """bf16 split terms for s = sum_c sta_c * (2 mov_c) - |mov|^2
        as seen with `sta` stationary; key_sq = -(|mov|^2)."""
        ta, tb = [], []
        for c in range(3):
            a, b = _split3(sta[:, c]), _split3(2.0 * mov[:, c])
            for i, j in ((0, 0), (0, 1), (0, 2), (1, 0), (1, 1), (2, 0)):
                ta.append(a[i])
                tb.append(b[j])
        sh, sm, sl = _split3(key_sq)
        ob = _to_bf16(ones)
        for s in (sh, sm, sl):
            ta.append(ob)
            tb.append(s)
        A = np.stack(ta).astype(ml_dtypes.bfloat16)
        Bm = np.stack(tb).astype(ml_dtypes.bfloat16)
        return A, Bm

    y2 = -(yb * yb).sum(-1)
    x2 = -(xb * xb).sum(-1)
    A1, B1 = build(xb, yb, y2)   # dir 1: lhsT = x terms, rhs = y terms
    A2, B2 = build(yb, xb, x2)   # dir 2: lhsT = y terms, rhs = x terms

    augs = np.empty((KSPLIT, 4 * n), ml_dtypes.bfloat16)
    augs[:, 0 * n:1 * n] = A1
    augs[:, 1 * n:2 * n] = B1
    augs[:, 2 * n:3 * n] = A2
    augs[:, 3 * n:4 * n] = B2

    iota = np.arange(R, dtype=np.float32)
    consts = np.tile(np.concatenate([iota, iota + R])[None, :], (P, 1))
    return {"augs": augs, "xr": xb, "yr": yb,
            "consts": np.ascontiguousarray(consts, np.float32)}


_NC = None


def _get_nc():
    global _NC
    if _NC is None:
        _NC = _build_nc()
    return _NC


def run_on_hw(x, y, **spmd_kwargs):
    """Run the SPMD kernel; returns (per-core out arrays, BassKernelResults)."""
    x = np.asarray(x, dtype=np.float32)
    y = np.asarray(y, dtype=np.float32)
    assert x.shape == (B, N_FULL, 3) and y.shape == (B, N_FULL, 3)
    nc = _get_nc()
    in_maps = [_host_prep(x[b], y[b]) for b in range(B)]
    res = bass_utils.run_bass_kernel_spmd(
        nc, in_maps, core_ids=list(range(B)), **spmd_kwargs)
    outs = [res.results[b]["out_s"] for b in range(B)]
    return outs, res


def kernel(x, y):
    outs, _ = run_on_hw(x, y)
    vals = []
    for o in outs:
        s = np.asarray(o, dtype=np.float64).sum(axis=0)
        vals.append(s[0] ** 0.2 + s[1] ** 0.2)
    return np.float32(np.mean(vals))


# revision 21
# speedup vs baseline: 2.4791x; 1.0008x over previous
"""Chamfer p=5 loss (nn_ChamferLossP) — Bass kernel for 8x TRN2 NeuronCores.

Sharding: data-parallel over the batch dim B=8, one batch per core; host
combines the per-core partial sums (the final "mean all-reduce").

Per-core device algorithm (direction 1 shown; direction 2 swaps x<->y):

  argmin_m ||x_n - y_m||^2  ==  argmax_m s[n,m],  s = 2 x.y - |y_m|^2.

  The PE materialises s in PSUM tiles [128n x 512m] with a single bf16
  matmul per tile: each fp32 factor is split into 3 bf16 limbs and the
  6 significant limb products per coordinate (plus 3 limbs of the -|y|^2
  term) form a 21-term contraction — fp32-accurate keys (~1e-7 rel) at
  bf16 speed (4x faster than the PE's multi-pass fp32 mode).

  Index extraction is two-level: the DVE reduces each PSUM tile to
  16-element group maxima (1 elem/cycle), then a short tensor_tensor_scan
  (running max, 2 cycles/elem but only N/16 elems) produces the prefix
  maxima r8 whose last column is the row max g.  The Scalar engine's
  Sign(g - r8) with accum_out counts groups strictly before the first
  attainment of g — the exact first-attainment group index (Sign(0)=0 on
  HW, probed).  One indirect DMA per 128-row chunk gathers that group's
  16 candidate points (48 contiguous floats); the epilogue recomputes the
  16 exact fp32 squared distances, picks the winner (first index on
  ties, matching jnp.argmin), and accumulates sum_c |x - nn|^5.
"""

import numpy as np
import ml_dtypes

import concourse.bass as bass
import concourse.bacc as bacc
import concourse.mybir as mybir
from concourse import bass_utils
from concourse.tile import TileContext

F32 = mybir.dt.float32
BF16 = mybir.dt.bfloat16
AF = mybir.ActivationFunctionType
ALU = mybir.AluOpType

B = 8
N_FULL = 4096
HALF_FULL = 2048
P = 128
R = 16              # argmin group size (candidates per gather)
KSPLIT = 21         # bf16 split-contraction terms
NEG_BIG = -3.0e38


def _build_nc(N=N_FULL, HALF=HALF_FULL, num_devices=B):
    NCH = N // P         # 128-row chunks per direction
    MMFD = min(512, HALF)
    NH = N // HALF       # psum tiles per chunk
    NG = N // R          # groups per row
    GH = HALF // R       # groups per psum tile

    nc = bacc.Bacc("TRN2", target_bir_lowering=False,
                   num_devices=num_devices)

    # augs columns: [x1_lhsT | y1_rhs | y2_lhsT | x2_rhs], each N wide, bf16.
    augs = nc.dram_tensor("augs", [KSPLIT, 4 * N], BF16,
                          kind="ExternalInput").ap()
    xr = nc.dram_tensor("xr", [N, 3], F32, kind="ExternalInput").ap()
    yr = nc.dram_tensor("yr", [N, 3], F32, kind="ExternalInput").ap()
    # consts row: [iota16 | iota16 + R]
    consts = nc.dram_tensor("consts", [P, 2 * R], F32,
                            kind="ExternalInput").ap()
    out_s = nc.dram_tensor("out_s", [P, 2], F32, kind="ExternalOutput").ap()

    with TileContext(nc) as tc:
        with (
            tc.tile_pool(name="const", bufs=1) as const_pool,
            tc.tile_pool(name="u", bufs=3) as u_pool,
            tc.tile_pool(name="r8", bufs=3) as r8_pool,
            tc.tile_pool(name="sgn", bufs=3) as sgn_pool,
            tc.tile_pool(name="idx", bufs=1) as idx_pool,
            tc.tile_pool(name="epi", bufs=1) as epi_pool,
            tc.tile_pool(name="psum", bufs=2, space="PSUM") as psum_pool,
        ):
            augs_sb = const_pool.tile([KSPLIT, 4 * N], BF16, tag="augs")
            # split the load so direction 1's operands arrive first
            for i in range(4):
                nc.sync.dma_start(augs_sb[:, i * N:(i + 1) * N],
                                  augs[:, i * N:(i + 1) * N])

            def aug(i):
                return augs_sb[:, i * N:(i + 1) * N]

            consts_sb = const_pool.tile([P, 2 * R], F32, tag="consts")
            nc.sync.dma_start(consts_sb[:], consts)

            dummy = const_pool.tile([P, 1], F32, tag="dummy")
            nc.vector.memset(dummy[:], 0.0)

            idxg_f = {1: idx_pool.tile([P, NCH], F32, tag="ig1", name="ig1"),
                      2: idx_pool.tile([P, NCH], F32, tag="ig2", name="ig2")}
            idxg_i = {1: idx_pool.tile([P, NCH], mybir.dt.int32, tag="ii1",
                                       name="ii1"),
                      2: idx_pool.tile([P, NCH], mybir.dt.int32, tag="ii2",
                                       name="ii2")}
            # gathered candidate groups, flat [P, NCH * R * 3]
            cand = {1: epi_pool.tile([P, NCH * R * 3], F32, tag="cand1",
                                     name="cand1"),
                    2: epi_pool.tile([P, NCH * R * 3], F32, tag="cand2",
                                     name="cand2")}

            for dirn in (1, 2):
                lhsT_all = aug(0) if dirn == 1 else aug(2)
                rhs_all = aug(1) if dirn == 1 else aug(3)
                gsrc = yr if dirn == 1 else xr
                gsrc_g = gsrc.rearrange("(g k) d -> g (k d)", k=R)
                for c in range(NCH):
                    r8 = r8_pool.tile([P, NG], F32, tag="r8")
                    u = u_pool.tile([P, NG], F32, tag="u")
                    for h in range(NH):
                        ps = psum_pool.tile([P, HALF], F32, tag="ps",
                                            space="PSUM")
                        for k in range(HALF // MMFD):
                            m0 = h * HALF + k * MMFD
                            nc.tensor.matmul(
                                ps[:, k * MMFD:(k + 1) * MMFD],
                                lhsT=lhsT_all[:, c * P:(c + 1) * P],
                                rhs=rhs_all[:, m0:m0 + MMFD],
                                start=True, stop=True,
                            )
                        # group maxima (R-wide) of this tile, 1 elem/cyc
                        nc.vector.tensor_reduce(
                            out=u[:, h * GH:(h + 1) * GH],
                            in_=ps[:].rearrange("p (g k) -> p g k", k=R),
                            axis=mybir.AxisListType.X,
                            op=ALU.max,
                        )
                    # prefix max over groups (2 cyc/elem, N/R elems)
                    nc.vector.tensor_tensor_scan(
                        out=r8[:],
                        data0=u[:],
                        data1=dummy[:, 0:1].to_broadcast([P, NG]),
                        initial=NEG_BIG,
                        op0=ALU.max,
                        op1=ALU.bypass,
                    )
                    # group index of first attainment of the row max
                    sgn = sgn_pool.tile([P, NG], BF16, tag="sgn")
                    nc.scalar.activation(
                        out=sgn[:], in_=r8[:, :],
                        func=AF.Sign,
                        bias=r8[:, NG - 1:NG],
                        scale=-1.0,
                        accum_out=idxg_f[dirn][:, c:c + 1],
                    )
                    nc.scalar.activation(
                        out=idxg_i[dirn][:, c:c + 1],
                        in_=idxg_f[dirn][:, c:c + 1],
                        func=AF.Copy, bias=0.0, scale=1.0)
                    # gather the 16-candidate group (48 contiguous floats)
                    nc.gpsimd.indirect_dma_start(
                        out=cand[dirn][:, c * R * 3:(c + 1) * R * 3],
                        out_offset=None,
                        in_=gsrc_g,
                        in_offset=bass.IndirectOffsetOnAxis(
                            ap=idxg_i[dirn][:, c:c + 1], axis=0),
                    )

            # ---- epilogue: exact within-group argmin + sum |diff|^5 ----
            partials = epi_pool.tile([P, 2], F32, tag="partials")
            FC = NCH * R * 3   # candidate floats per partition
            FK = NCH * R       # candidates per partition
            for dirn in (1, 2):
                own = xr if dirn == 1 else yr
                cd = cand[dirn]

                ow = epi_pool.tile([P, NCH, 3], F32, tag=f"ow{dirn}",
                                   name=f"ow{dirn}")
                nc.sync.dma_start(
                    ow[:], own.rearrange("(c p) d -> p c d", p=P))
                # own broadcast over the R candidates: [P, NCH, R, 3]
                owb = bass.AP(ow[:].tensor, ow[:].offset,
                              [ow[:].ap[0], [3, NCH], [0, R], [1, 3]])

                diff = epi_pool.tile([P, FC], F32, tag=f"df{dirn}",
                                     name=f"df{dirn}")
                nc.vector.tensor_sub(
                    diff[:].rearrange("p (c k d) -> p c k d", k=R, d=3),
                    owb, cd[:].rearrange("p (c k d) -> p c k d", k=R, d=3))
                ad = epi_pool.tile([P, FC], F32, tag=f"ab{dirn}",
                                   name=f"ab{dirn}")
                nc.scalar.activation(out=ad[:], in_=diff[:], func=AF.Abs,
                                     bias=0.0, scale=1.0)
                sq = epi_pool.tile([P, FC], F32, tag=f"sq{dirn}",
                                   name=f"sq{dirn}")
                nc.scalar.activation(out=sq[:], in_=ad[:], func=AF.Square,
                                     bias=0.0, scale=1.0)
                # squared L2 distance per candidate [P, NCH*R]
                dd = epi_pool.tile([P, FK], F32, tag=f"dd{dirn}",
                                   name=f"dd{dirn}")
                nc.vector.tensor_reduce(
                    out=dd[:], in_=sq[:].rearrange("p (k d) -> p k d", d=3),
                    axis=mybir.AxisListType.X, op=ALU.add)
                # min distance per row [P, NCH]
                dmin = epi_pool.tile([P, NCH], F32, tag=f"dm{dirn}",
                                     name=f"dm{dirn}")
                nc.vector.tensor_reduce(
                    out=dmin[:], in_=dd[:].rearrange("p (c k) -> p c k", k=R),
                    axis=mybir.AxisListType.X, op=ALU.min)
                dminb = bass.AP(dmin[:].tensor, dmin[:].offset,
                                [dmin[:].ap[0], [1, NCH], [0, R]])
                mask = epi_pool.tile([P, FK], F32, tag=f"mk{dirn}",
                                     name=f"mk{dirn}")
                nc.vector.tensor_tensor(
                    out=mask[:].rearrange("p (c k) -> p c k", k=R),
                    in0=dd[:].rearrange("p (c k) -> p c k", k=R),
                    in1=dminb, op=ALU.is_le)
                # first-attaining candidate: k* = min_k (iota_k + 16*(1-mask))
                iotap = bass.AP(consts_sb[:].tensor, consts_sb[:].offset + R,
                               [consts_sb[:].ap[0], [0, NCH], [1, R]])
                tk = epi_pool.tile([P, FK], F32, tag=f"tk{dirn}",
                                   name=f"tk{dirn}")
                nc.vector.scalar_tensor_tensor(
                    out=tk[:].rearrange("p (c k) -> p c k", k=R),
                    in0=mask[:].rearrange("p (c k) -> p c k", k=R),
                    scalar=-float(R), op0=ALU.mult,
                    in1=iotap, op1=ALU.add)
                kstar = epi_pool.tile([P, NCH], F32, tag=f"ks{dirn}",
                                      name=f"ks{dirn}")
                nc.vector.tensor_reduce(
                    out=kstar[:], in_=tk[:].rearrange("p (c k) -> p c k", k=R),
                    axis=mybir.AxisListType.X, op=ALU.min)
                # tk = (iota+R) - R*mask: attaining entries are exactly
                # iota_k, others iota_k+R, so min over k is k*.
                ksb = bass.AP(kstar[:].tensor, kstar[:].offset,
                              [kstar[:].ap[0], [1, NCH], [0, R]])
                onehot = epi_pool.tile([P, FK], F32, tag=f"oh{dirn}",
                                       name=f"oh{dirn}")
                nc.vector.tensor_tensor(
                    out=onehot[:].rearrange("p (c k) -> p c k", k=R),
                    in0=bass.AP(consts_sb[:].tensor, consts_sb[:].offset,
                                [consts_sb[:].ap[0], [0, NCH], [1, R]]),
                    in1=ksb, op=ALU.is_equal)
                # p5 per candidate, then select the winner
                q4 = epi_pool.tile([P, FC], F32, tag=f"q4{dirn}",
                                   name=f"q4{dirn}")
                nc.scalar.activation(out=q4[:], in_=sq[:], func=AF.Square,
                                     bias=0.0, scale=1.0)
                p5e = epi_pool.tile([P, FC], F32, tag=f"p5{dirn}",
                                    name=f"p5{dirn}")
                nc.gpsimd.tensor_mul(p5e[:], q4[:], ad[:])
                p5k = epi_pool.tile([P, FK], F32, tag=f"pk{dirn}",
                                    name=f"pk{dirn}")
                nc.vector.tensor_reduce(
                    out=p5k[:], in_=p5e[:].rearrange("p (k d) -> p k d", d=3),
                    axis=mybir.AxisListType.X, op=ALU.add)
                psel = epi_pool.tile([P, FK], F32, tag=f"pl{dirn}",
                                     name=f"pl{dirn}")
                nc.gpsimd.tensor_mul(psel[:], p5k[:], onehot[:])
                nc.vector.reduce_sum(partials[:, dirn - 1:dirn], psel[:],
                                     axis=mybir.AxisListType.X)

            nc.sync.dma_start(out_s, partials[:])

    nc.compile()
    return nc


def _to_bf16(a):
    return a.astype(ml_dtypes.bfloat16)


def _split3(a):
    a = np.asarray(a, np.float32)
    h = _to_bf16(a)
    m = _to_bf16(a - h.astype(np.float32))
    l = _to_bf16(a - h.astype(np.float32) - m.astype(np.float32))
    return h, m, l


def _host_prep(xb, yb):
    xb = np.ascontiguousarray(xb, dtype=np.float32)
    yb = np.ascontiguousarray(yb, dtype=np.float32)
    n = xb.shape[0]
    ones = np.ones((n,), np.float32)

    def build(sta, mov, key_sq):
        """bf16 split terms for s = sum_c sta_c * (2 mov_c) - |mov|^2
        as seen with `sta` stationary; key_sq = -(|mov|^2)."""
        ta, tb = [], []
        for c in range(3):
            a, b = _split3(sta[:, c]), _split3(2.0 * mov[:, c])
            for i, j in ((0, 0), (0, 1), (0, 2), (1, 0), (1, 1), (2, 0)):
                ta.append(a[i])
                tb.append(b[j])
        sh, sm, sl = _split3(key_sq)
        ob = _to_bf16(ones)
        for s in (sh, sm, sl):
            ta.append(ob)
            tb.append(s)
        A = np.stack(ta).astype(ml_dtypes.bfloat16)
        Bm = np.stack(tb).astype(ml_dtypes.bfloat16)
        return A, Bm

    y2 = -(yb * yb).sum(-1)
    x2 = -(xb * xb).sum(-1)
    A1, B1 = build(xb, yb, y2)   # dir 1: lhsT = x terms, rhs = y terms
    A2, B2 = build(yb, xb, x2)   # dir 2: lhsT = y terms, rhs = x terms

    augs = np.empty((KSPLIT, 4 * n), ml_dtypes.bfloat16)
    augs[:, 0 * n:1 * n] = A1
    augs[:, 1 * n:2 * n] = B1
    augs[:, 2 * n:3 * n] = A2
    augs[:, 3 * n:4 * n] = B2

    iota = np.arange(R, dtype=np.float32)
    consts = np.tile(np.concatenate([iota, iota + R])[None, :], (P, 1))
    return {"augs": augs, "xr": xb, "yr": yb,
            "consts": np.ascontiguousarray(consts, np.float32)}


_NC = None


def _get_nc():
    global _NC
    if _NC is None:
        _NC = _build_nc()
    return _NC


def run_on_hw(x, y, **spmd_kwargs):
    """Run the SPMD kernel; returns (per-core out arrays, BassKernelResults)."""
    x = np.asarray(x, dtype=np.float32)
    y = np.asarray(y, dtype=np.float32)
    assert x.shape == (B, N_FULL, 3) and y.shape == (B, N_FULL, 3)
    nc = _get_nc()
    in_maps = [_host_prep(x[b], y[b]) for b in range(B)]
    res = bass_utils.run_bass_kernel_spmd(
        nc, in_maps, core_ids=list(range(B)), **spmd_kwargs)
    outs = [res.results[b]["out_s"] for b in range(B)]
    return outs, res


def kernel(x, y):
    outs, _ = run_on_hw(x, y)
    vals = []
    for o in outs:
        s = np.asarray(o, dtype=np.float64).sum(axis=0)
        vals.append(s[0] ** 0.2 + s[1] ** 0.2)
    return np.float32(np.mean(vals))
